# revision 68
# baseline (speedup 1.0000x reference)
"""Trainium2 Bass kernel for the dual-stream encoder block.

Sharding: 8 cores = 4 batches x 2 query-row halves (2048 rows/core).
Inputs are pre-rolled along L per core so output rows are always 0..2047;
K/V contraction uses the full 4096 rows. No cross-core communication.
"""

import sys

sys.path.insert(0, "/opt/trn_rl_repo")

import numpy as np
import ml_dtypes

B, L, D, OUT = 4, 4096, 128, 55
D2, H = 256, 512
A = 2048  # output rows per core
NT = 32  # l-tiles of 128
NG = 8  # l-groups of 4 tiles
AT = 16  # a-tiles per core
AC = 4  # a-chunks of 512
SCALE = float(1.0 / np.sqrt(np.float32(128.0)))

_CACHE = {}


def _build_nc():
    import concourse.bass as bass
    from concourse import bacc, mybir
    import concourse.tile as tile
    from concourse.masks import make_identity

    f32 = mybir.dt.float32
    bf16 = mybir.dt.bfloat16
    AF = mybir.ActivationFunctionType
    ALU = mybir.AluOpType

    fp8 = mybir.dt.float8e4

    nc = bacc.Bacc("TRN2", target_bir_lowering=False, debug=False)

    # ---- DRAM parameters -------------------------------------------------
    dx1 = nc.dram_tensor("x1", [128, NT, D], f32, kind="ExternalInput")
    dx2 = nc.dram_tensor("x2", [128, NT, D], f32, kind="ExternalInput")
    dres1 = nc.dram_tensor("res1p", [128, AT, D], f32, kind="ExternalInput")
    dres2 = nc.dram_tensor("res2p", [128, AT, D], f32, kind="ExternalInput")
    dwpack = nc.dram_tensor("wpack", [128, 878], bf16, kind="ExternalInput")
    dwf18 = nc.dram_tensor("wf18", [128, 1024], fp8, kind="ExternalInput")
    dwf28 = nc.dram_tensor("wf28", [128, 1024], fp8, kind="ExternalInput")
    dvpack = nc.dram_tensor("vpack", [128, 6], f32, kind="ExternalInput")
    dbf2 = nc.dram_tensor("bf2", [D2], f32, kind="ExternalInput")
    dbo = nc.dram_tensor("bo", [OUT], f32, kind="ExternalInput")
    dzpad = nc.dram_tensor("zpad", [128, L], fp8, kind="ExternalInput")
    dout = nc.dram_tensor("out", [128, AT, OUT], f32, kind="ExternalOutput")

    def bcast_ap(dt_handle, n):
        ap = dt_handle.ap()
        return bass.AP(tensor=ap.tensor, offset=ap.offset, ap=[[0, 128], [1, n]])

    with tile.TileContext(nc) as tc:
        import contextlib

        with contextlib.ExitStack() as ctx:
            consts = ctx.enter_context(tc.tile_pool(name="consts", bufs=1))
            big = ctx.enter_context(tc.tile_pool(name="big", bufs=1))
            stats = ctx.enter_context(tc.tile_pool(name="stats", bufs=3))
            xpool = ctx.enter_context(tc.tile_pool(name="xpool", bufs=4))
            xspool = ctx.enter_context(tc.tile_pool(name="xs", bufs=5))
            respool = ctx.enter_context(tc.tile_pool(name="res", bufs=1))
            outpool = ctx.enter_context(tc.tile_pool(name="outp", bufs=1))

            DR = mybir.MatmulPerfMode.DoubleRow

            # ---- constants ----
            ident = consts.tile([128, 128], bf16)
            make_identity(nc, ident[:])
            ones8 = consts.tile([128, 2, 1], fp8)
            nc.vector.memset(ones8[:], 1.0)

            wpk = consts.tile([128, 878], bf16)
            wf18 = consts.tile([128, 2, 4, 128], fp8)
            wf28 = consts.tile([128, 2, 2, 256], fp8)
            vpk = consts.tile([128, 6], f32)
            wq = wpk[:, 0:128]
            wk = wpk[:, 128:256]
            wv1 = wpk[:, 256:384]
            wv2 = wpk[:, 384:512]
            wp1 = wpk[:, 512:640]
            wp2 = wpk[:, 640:768]
            wov = lambda sh: wpk[:, 768 + 55 * sh : 768 + 55 * (sh + 1)]
            bq = vpk[:, 0:1]
            bk = vpk[:, 1:2]
            bf1t = vpk[:, 2:6]
            bf2b = consts.tile([128, D2], f32)
            bob = consts.tile([128, OUT], f32)

            # ---- big SBUF residents ----
            x1nT = big.tile([128, L], bf16)  # [d, l]
            x2nT = big.tile([128, L], bf16)
            q1T8 = big.tile([128, 2, L], fp8)  # [d, (real|zero), l] x32
            k2T8 = big.tile([128, 2, A], fp8)
            v1_8 = big.tile([128, NT, 128], fp8)  # [l-part, tile, d] x32
            v2_8 = big.tile([128, NT, 128], fp8)
            attF = [
                big.tile([128, NT, 512], fp8, name="attF0", tag="attF0"),
                big.tile([128, NT, 512], fp8, name="attF1", tag="attF1"),
            ]
            o1T = big.tile([128, A], bf16)  # (att @ v1)^T, unnorm, /32
            o2T = big.tile([128, A], bf16)
            invd = big.tile([128, AT], f32)
            xcat = big.tile([128, AT, D2], f32)
            xfT8 = big.tile([128, 2, A], fp8)  # lnf normalized^T x16
            x3Tl = big.tile([128, A], bf16)  # ln3 normalized^T halves
            x3Th = big.tile([128, A], bf16)
            h1T8 = big.tile([128, 4, A], fp8)

            # =========== Phase A (LN+transpose+QKV) interleaved with =====
            # =========== chunk-0 attention ===============================
            psB_cm = tc.tile_pool(name="psB", bufs=2, space="PSUM")
            psB = psB_cm.__enter__()
            psBo_cm = tc.tile_pool(name="psBo", bufs=1, space="PSUM")
            psBo = psBo_cm.__enter__()
            psA_cm = tc.tile_pool(name="psA", bufs=2, space="PSUM")
            psA = psA_cm.__enter__()

            BSs = [stats.tile([128, NT, 6], f32, tag="BSs%d" % s, name="BSs%d" % s) for s in (0, 1)]
            MVs = [stats.tile([128, NT, 2], f32, tag="MVs%d" % s, name="MVs%d" % s) for s in (0, 1)]
            RSs = [stats.tile([128, NT], f32, tag="RSs%d" % s, name="RSs%d" % s) for s in (0, 1)]
            nAs = [stats.tile([128, NT], f32, tag="nAs%d" % s, name="nAs%d" % s) for s in (0, 1)]
            nBs = [stats.tile([128, NT], f32, tag="nBs%d" % s, name="nBs%d" % s) for s in (0, 1)]
            BSf = stats.tile([128, AT, 6], f32, tag="BSf")
            MVf = stats.tile([128, AT, 2], f32, tag="MVf")
            RSf = stats.tile([128, AT], f32, tag="RSf")
            nt1 = stats.tile([128, AT], f32, tag="nt1")
            nt2 = stats.tile([128, AT], f32, tag="nt2")

            def newton_rsqrt(dst, var_ap, t1, t2, scale16=False, iters=3):
                # dst = rsqrt(var) [x16] on Pool; 1 + `iters` Newton steps,
                # var in ~[0.5, 2]; the x16 rides the last iteration free.
                nc.gpsimd.tensor_scalar(
                    dst, var_ap, -0.5, 1.5, op0=ALU.mult, op1=ALU.add
                )
                for it in range(iters):
                    last = it == iters - 1
                    c0, c1 = (-8.0, 24.0) if (last and scale16) else (-0.5, 1.5)
                    nc.gpsimd.tensor_tensor(t1, dst, dst, op=ALU.mult)
                    nc.gpsimd.tensor_tensor(t2, var_ap, t1, op=ALU.mult)
                    nc.gpsimd.tensor_scalar(
                        t1, t2, c0, c1, op0=ALU.mult, op1=ALU.add
                    )
                    nc.gpsimd.tensor_tensor(dst, dst, t1, op=ALU.mult)

            def ln_to_T(MVx, RSx, chunks, norm_eng, evac_act=False,
                        pool=None, tag="tr2"):
                # xcat rows -> x16-normalized fp8 transposed halves in xfT8
                for jj in chunks:
                    psT2 = (pool or psC).tile([128, 4, 256], bf16, tag=tag)
                    for k in range(4):
                        t = 4 * jj + k
                        xsf = xspool.tile([128, D2], bf16, tag="xsf")
                        norm_eng.tensor_scalar(
                            xsf[:],
                            xcat[:, t, :],
                            MVx[:, t, 0:1],
                            RSx[:, t : t + 1],
                            op0=ALU.subtract,
                            op1=ALU.mult,
                        )
                        nc.tensor.transpose(psT2[:, k, 0:128], xsf[:, 0:128], ident[:])
                        nc.tensor.transpose(psT2[:, k, 128:256], xsf[:, 128:256], ident[:])
                    if evac_act:
                        nc.scalar.activation(
                            xfT8[:, 0, 512 * jj : 512 * (jj + 1)],
                            psT2[:, :, 0:128], AF.Identity,
                        )
                        nc.scalar.activation(
                            xfT8[:, 1, 512 * jj : 512 * (jj + 1)],
                            psT2[:, :, 128:256], AF.Identity,
                        )
                    else:
                        nc.vector.tensor_copy(
                            xfT8[:, 0, 512 * jj : 512 * (jj + 1)], psT2[:, :, 0:128]
                        )
                        nc.vector.tensor_copy(
                            xfT8[:, 1, 512 * jj : 512 * (jj + 1)], psT2[:, :, 128:256]
                        )


            Xhold = {}
            dxs = (dx1, dx2)
            xnTs = (x1nT, x2nT)

            def s_stats(s, g):
                X = xpool.tile([128, 4, 128], f32, tag="X%d" % s)
                nc.sync.dma_start(X[:], dxs[s].ap()[:, 4 * g : 4 * g + 4, :])
                for k in range(4):
                    i = 4 * g + k
                    nc.vector.bn_stats(BSs[s][:, i, :], X[:, k, :])
                    nc.vector.bn_aggr(MVs[s][:, i, :], BSs[s][:, i, :])
                Xhold[(s, g)] = X

            def s_xform(s, g):
                X = Xhold.pop((s, g))
                for k in range(4):
                    i = 4 * g + k
                    xs = xspool.tile([128, 128], bf16, tag="xs")
                    nc.gpsimd.tensor_scalar(
                        xs[:], X[:, k, :], MVs[s][:, i, 0:1], RSs[s][:, i : i + 1],
                        op0=ALU.subtract, op1=ALU.mult,
                    )
                    if k == 0:
                        psT = psA.tile([128, 4, 128], bf16, tag="ps")
                    nc.tensor.transpose(psT[:, k, :], xs[:], ident[:])
                nc.vector.tensor_copy(
                    xnTs[s][:, 512 * g : 512 * (g + 1)], psT[:]
                )

            def qkv1(g):
                psQ = psA.tile([128, 512], f32, tag="ps")
                nc.tensor.matmul(
                    psQ[:], wq, x1nT[:, 512 * g : 512 * (g + 1)],
                    start=True, stop=True,
                )
                nc.vector.tensor_scalar(
                    q1T8[:, 0, 512 * g : 512 * (g + 1)], psQ[:], 32.0, bq,
                    op0=ALU.mult, op1=ALU.add,
                )
                psV = psA.tile([128, 512], f32, tag="ps")
                for k in range(4):
                    i = 4 * g + k
                    nc.tensor.matmul(
                        psV[:, 128 * k : 128 * (k + 1)],
                        x1nT[:, 128 * i : 128 * (i + 1)], wv1,
                        start=True, stop=True,
                    )
                nc.vector.tensor_scalar(
                    v1_8[:, 4 * g : 4 * g + 4, :], psV[:], 32.0, None, op0=ALU.mult
                )

            def qkv2(g):
                if g < 4:
                    psK = psA.tile([128, 512], f32, tag="ps")
                    nc.tensor.matmul(
                        psK[:], wk, x2nT[:, 512 * g : 512 * (g + 1)],
                        start=True, stop=True,
                    )
                    nc.vector.tensor_scalar(
                        k2T8[:, 0, 512 * g : 512 * (g + 1)], psK[:], 32.0, bk,
                        op0=ALU.mult, op1=ALU.add,
                    )
                psV = psA.tile([128, 512], f32, tag="ps")
                for k in range(4):
                    i = 4 * g + k
                    nc.tensor.matmul(
                        psV[:, 128 * k : 128 * (k + 1)],
                        x2nT[:, 128 * i : 128 * (i + 1)], wv2,
                        start=True, stop=True,
                    )
                nc.scalar.activation(
                    v2_8[:, 4 * g : 4 * g + 4, :], psV[:], AF.Identity, scale=32.0
                )

            class Chunk:
                def __init__(self, j):
                    self.j = j
                    self.psO1 = psBo.tile([128, 512], f32, tag="o1")
                    self.psO2 = psBo.tile([128, 512], f32, tag="o2")
                    self.aF = attF[j % 2]

                def att_pair(self, g, sl=slice(None)):
                    return self.aF[:, 2 * g : 2 * g + 2, sl]

                def ex_pair(self, g):
                    psE = psB.tile([128, 2, 512], f32, tag="e")
                    for k in range(2):
                        i = 2 * g + k
                        nc.tensor.matmul(
                            psE[:, k, :],
                            q1T8[:, :, 128 * i : 128 * (i + 1)],
                            k2T8[:, :, 512 * self.j : 512 * (self.j + 1)],
                            start=True, stop=True, perf_mode=DR,
                        )
                    nc.scalar.activation(
                        self.att_pair(g), psE[:], AF.Exp, scale=SCALE / 1024.0
                    )

                def av_pair(self, g):
                    ap_ = self.att_pair(g)
                    last = g == NT // 2 - 1
                    nc.tensor.matmul(
                        self.psO1[:], v1_8[:, 2 * g : 2 * g + 2, :], ap_,
                        start=(g == 0), stop=last,
                        perf_mode=DR, skip_group_check=True,
                    )
                    nc.tensor.matmul(
                        self.psO2[:], v2_8[:, 2 * g : 2 * g + 2, :], ap_,
                        start=(g == 0), stop=last,
                        perf_mode=DR, skip_group_check=True,
                    )

                def den_pair(self, g, psDen):
                    last = g == NT // 2 - 1
                    for t in range(4):
                        nc.tensor.matmul(
                            psDen[:, t : t + 1],
                            self.att_pair(g, slice(128 * t, 128 * (t + 1))),
                            ones8[:],
                            start=(g == 0), stop=last,
                            perf_mode=DR, skip_group_check=True,
                        )

            # ---- interleaved head: 4 rounds x (2 groups/stream + pairs) ---
            # chunk-0 pairs ride each round; chunk-1 exps lag one round
            c0 = Chunk(0)
            c1 = Chunk(1)
            for m in range(4):
                for s in (0, 1):
                    s_stats(s, 2 * m)
                    s_stats(s, 2 * m + 1)
                if m == 0:
                    # weights + zero-pad halves load behind the first X DMAs
                    nc.scalar.dma_start(wpk[:], dwpack[:])
                    nc.scalar.dma_start(vpk[:], dvpack[:])
                    nc.sync.dma_start(q1T8[:, 1, :], dzpad[:])
                    nc.sync.dma_start(k2T8[:, 1, :], dzpad.ap()[:, 0:A])
                if m == 0:
                    for s in (0, 1):
                        for gg in (0, 1):
                            sl4 = slice(4 * gg, 4 * gg + 4)
                            newton_rsqrt(
                                RSs[s][:, sl4], MVs[s][:, sl4, 1],
                                nAs[s][:, sl4], nBs[s][:, sl4], iters=2,
                            )
                else:
                    sl8 = slice(8 * m, 8 * m + 8)
                    for s in (0, 1):
                        newton_rsqrt(
                            RSs[s][:, sl8], MVs[s][:, sl8, 1],
                            nAs[s][:, sl8], nBs[s][:, sl8], iters=2,
                        )
                for s in (0, 1):
                    s_xform(s, 2 * m)
                    s_xform(s, 2 * m + 1)
                qkv1(2 * m)
                qkv2(2 * m)
                qkv1(2 * m + 1)
                qkv2(2 * m + 1)
                for p in range(4 * m, 4 * m + 4):
                    c0.ex_pair(p)
                    if p > 0:
                        c0.av_pair(p - 1)
                    if m > 0:
                        c1.ex_pair(p - 4)
            c0.av_pair(NT // 2 - 1)
            for p in range(12, NT // 2):
                c1.ex_pair(p)

            psA_cm.__exit__(None, None, None)
            psD_cm = tc.tile_pool(name="psD", bufs=1, space="PSUM")
            psD = psD_cm.__enter__()
            psDen = psD.tile([128, AC, 4], f32, tag="den", name="psDen")

            r1 = respool.tile([128, AT, D], f32, tag="r1")
            r2 = respool.tile([128, AT, D], f32, tag="r2")
            r_loaded = set()

            def load_res(j):
                if j in r_loaded:
                    return
                r_loaded.add(j)
                nc.sync.dma_start(
                    r1[:, 4 * j : 4 * j + 4, :], dres1.ap()[:, 4 * j : 4 * j + 4, :]
                )
                nc.sync.dma_start(
                    r2[:, 4 * j : 4 * j + 4, :], dres2.ap()[:, 4 * j : 4 * j + 4, :]
                )

            def evac_chunk(c):
                j = c.j
                nc.vector.reciprocal(invd[:, 4 * j : 4 * j + 4], psDen[:, j, :])
                nc.vector.tensor_scalar(
                    o1T[:, 512 * j : 512 * (j + 1)], c.psO1[:], 1.0 / 32.0, None,
                    op0=ALU.mult,
                )
                nc.vector.tensor_scalar(
                    o2T[:, 512 * j : 512 * (j + 1)], c.psO2[:], 1.0 / 32.0, None,
                    op0=ALU.mult,
                )

            def proj_chunk(j):
                # output projection + residual + lnf stats for this chunk
                for k in range(4):
                    t = 4 * j + k
                    for (oT, wp, rr, off) in (
                        (o1T, wp1, r1[:, t, :], 0),
                        (o2T, wp2, r2[:, t, :], D),
                    ):
                        psP = psD.tile([128, D], f32, tag="p")
                        nc.tensor.matmul(
                            psP[:], oT[:, 128 * t : 128 * (t + 1)], wp,
                            start=True, stop=True,
                        )
                        sc = xspool.tile([128, D], f32, tag="sc")
                        nc.vector.tensor_scalar(
                            sc[:], psP[:], invd[:, t : t + 1], None, op0=ALU.mult
                        )
                        nc.gpsimd.tensor_tensor(
                            xcat[:, t, off : off + D], sc[:], rr, op=ALU.add
                        )
                    nc.vector.bn_stats(BSf[:, t, :], xcat[:, t, :])
                    nc.vector.bn_aggr(MVf[:, t, :], BSf[:, t, :])
                sl = slice(4 * j, 4 * (j + 1))
                newton_rsqrt(
                    RSf[:, sl], MVf[:, sl, 1], nt1[:, sl], nt2[:, sl], scale16=True
                )

            for g in range(NT // 2):
                c0.den_pair(g, psDen[:, 0, :])
            evac_chunk(c0)
            for g in range(NT // 2):
                c1.av_pair(g)
                c1.den_pair(g, psDen[:, 1, :])
            evac_chunk(c1)

            pend = [0, 1]
            for j in range(2, AC):
                c = Chunk(j)
                if j == 2:
                    # deferred FFN constants (transfer hides under attention)
                    nc.scalar.dma_start(wf18[:], dwf18[:])
                    nc.scalar.dma_start(wf28[:], dwf28[:])
                    nc.scalar.dma_start(bf2b[:], bcast_ap(dbf2, D2))
                    nc.scalar.dma_start(bob[:], bcast_ap(dbo, OUT))
                load_res(pend[0])
                if len(pend) > 1:
                    load_res(pend[1])
                for g in range(NT // 2):
                    c.ex_pair(g)
                    if g > 0:
                        c.av_pair(g - 1)
                        c.den_pair(g - 1, psDen[:, j, :])
                    if g in (5, 10) and pend:
                        proj_chunk(pend.pop(0))
                    if j == AC - 1 and g == 12:
                        # hoist lnf transpose of finished chunks under the
                        # last chunk's exp stream (psP bank is idle here)
                        ln_to_T(MVf, RSf, [0, 1, 2], norm_eng=nc.gpsimd,
                                pool=psD, tag="p")
                        for t in range(12):
                            nc.gpsimd.tensor_tensor(
                                xcat[:, t, :], xcat[:, t, :], bf2b[:],
                                op=ALU.add,
                            )
                c.av_pair(NT // 2 - 1)
                c.den_pair(NT // 2 - 1, psDen[:, j, :])
                evac_chunk(c)
                pend.append(j)
            # ---- FFN + outputs ----------------------------------------
            BS3 = stats.tile([128, AT, 6], f32, tag="BS3")
            MV3 = stats.tile([128, AT, 2], f32, tag="MV3")
            RS3 = stats.tile([128, AT], f32, tag="RS3")
            ov = dout.ap()
            osb = outpool.tile([128, AT, OUT], f32, tag="osb")

            def f1gelu(jj, pool, tag):
                for n in range(4):
                    psH = pool.tile([128, 512], f32, tag=tag, name="psH")
                    nc.tensor.matmul(
                        psH[:],
                        wf18[:, :, n, :],
                        xfT8[:, :, 512 * jj : 512 * (jj + 1)],
                        start=True,
                        stop=True,
                        perf_mode=DR,
                    )
                    nc.scalar.activation(
                        h1T8[:, n, 512 * jj : 512 * (jj + 1)],
                        psH[:],
                        AF.Gelu,
                        bias=bf1t[:, n : n + 1],
                        scale=1.0 / 1024.0,
                    )

            for j in pend:
                load_res(j)
            for j in pend:
                proj_chunk(j)
            ln_to_T(MVf, RSf, [3], norm_eng=nc.gpsimd, pool=psD, tag="p")
            for t in range(12, AT):
                nc.gpsimd.tensor_tensor(
                    xcat[:, t, :], xcat[:, t, :], bf2b[:], op=ALU.add
                )

            psD_cm.__exit__(None, None, None)
            psBo_cm.__exit__(None, None, None)
            psB_cm.__exit__(None, None, None)

            psC = ctx.enter_context(tc.tile_pool(name="psC", bufs=2, space="PSUM"))
            psHp = ctx.enter_context(tc.tile_pool(name="psHp", bufs=2, space="PSUM"))
            for jj in range(AC):
                f1gelu(jj, psHp, "h")

            def out_chunk(jj):
                # ln3-normalize chunk jj (bf16) and project to the output
                psT2 = psC.tile([128, 4, 256], bf16, tag="tr2", name="psT3")
                for k in range(4):
                    t = 4 * jj + k
                    xsf = xspool.tile([128, D2], bf16, tag="xsf")
                    nc.gpsimd.tensor_scalar(
                        xsf[:], xcat[:, t, :], MV3[:, t, 0:1], RS3[:, t : t + 1],
                        op0=ALU.subtract, op1=ALU.mult,
                    )
                    nc.tensor.transpose(psT2[:, k, 0:128], xsf[:, 0:128], ident[:])
                    nc.tensor.transpose(psT2[:, k, 128:256], xsf[:, 128:256], ident[:])
                nc.scalar.activation(
                    x3Tl[:, 512 * jj : 512 * (jj + 1)], psT2[:, :, 0:128],
                    AF.Identity,
                )
                nc.vector.tensor_copy(
                    x3Th[:, 512 * jj : 512 * (jj + 1)], psT2[:, :, 128:256]
                )
                for t in range(4 * jj, 4 * jj + 4):
                    psO = psC.tile([128, OUT], f32, tag="po", name="psOu")
                    nc.tensor.matmul(
                        psO[:], x3Tl[:, 128 * t : 128 * (t + 1)], wov(0),
                        start=True, stop=False, skip_group_check=True,
                    )
                    nc.tensor.matmul(
                        psO[:], x3Th[:, 128 * t : 128 * (t + 1)], wov(1),
                        start=False, stop=True, skip_group_check=True,
                    )
                    nc.vector.tensor_tensor(osb[:, t, :], psO[:], bob[:], op=ALU.add)
                nc.sync.dma_start(
                    ov[:, 4 * jj : 4 * jj + 4, :], osb[:, 4 * jj : 4 * jj + 4, :]
                )

            for jj in range(AC):
                for t in range(4 * jj, 4 * jj + 4):
                    psH2 = psC.tile([128, D2], f32, tag="h2", name="psH2")
                    for u in range(2):
                        nc.tensor.matmul(
                            psH2[:],
                            h1T8[:, 2 * u : 2 * u + 2, 128 * t : 128 * (t + 1)],
                            wf28[:, u, :, :],
                            start=(u == 0),
                            stop=(u == 1),
                            perf_mode=DR,
                            skip_group_check=True,
                        )
                    nc.vector.scalar_tensor_tensor(
                        xcat[:, t, :], psH2[:], 1.0 / 64.0, xcat[:, t, :],
                        op0=ALU.mult, op1=ALU.add,
                    )
                    nc.vector.bn_stats(BS3[:, t, :], xcat[:, t, :])
                    nc.vector.bn_aggr(MV3[:, t, :], BS3[:, t, :])
                sl = slice(4 * jj, 4 * jj + 4)
                newton_rsqrt(
                    RS3[:, sl], MV3[:, sl, 1], nt1[:, sl], nt2[:, sl], iters=2
                )
                if jj >= 1:
                    out_chunk(jj - 1)
            out_chunk(3)

    nc.compile()
    return nc


def _get_nc():
    if "nc" not in _CACHE:
        _CACHE["nc"] = _build_nc()
    return _CACHE["nc"]


def kernel(**inputs):
    from concourse.bass_utils import run_bass_kernel_spmd

    f = lambda k: np.asarray(inputs[k], dtype=np.float32)
    bf = lambda a: np.asarray(a, dtype=np.float32).astype(ml_dtypes.bfloat16)

    x1, x2 = f("x1"), f("x2")
    g1, b1 = f("ln1_g"), f("ln1_b")
    g2, b2 = f("ln2_g"), f("ln2_b")
    gf_, bf_ = f("lnf_g"), f("lnf_b")
    g3, b3 = f("ln3_g"), f("ln3_b")
    # fold LN gains/biases into the adjacent linear layers
    Wq = g1[:, None] * f("Wq"); bqp = b1 @ f("Wq") + f("bq")
    Wk = g2[:, None] * f("Wk"); bkp = b2 @ f("Wk") + f("bk")
    Wv1 = g1[:, None] * f("Wv1"); bv1p = b1 @ f("Wv1") + f("bv1")
    Wv2 = g2[:, None] * f("Wv2"); bv2p = b2 @ f("Wv2") + f("bv2")
    Wf1 = gf_[:, None] * f("Wf1"); bf1p = bf_ @ f("Wf1") + f("bf1")
    Wo = g3[:, None] * f("Wo"); bop = b3 @ f("Wo") + f("bo")
    Wp1, Wp2 = f("Wp1"), f("Wp2")
    bp1p = bv1p @ Wp1 + f("bp1")
    bp2p = bv2p @ Wp2 + f("bp2")

    Wf2 = f("Wf2")
    fp8 = lambda a: np.asarray(a, dtype=np.float32).astype(ml_dtypes.float8_e4m3)
    wpack = np.concatenate(
        [bf(Wq), bf(Wk), bf(Wv1), bf(Wv2), bf(Wp1), bf(Wp2),
         bf(Wo).reshape(2, 128, OUT).transpose(1, 0, 2).reshape(128, 2 * OUT)],
        axis=1,
    )
    # Wf1 [256,512] x64 -> [128, (kh, n, np)] fp8
    wf18 = fp8(
        (64.0 * Wf1).reshape(2, 128, 4, 128).transpose(1, 0, 2, 3).reshape(128, 1024)
    )
    # Wf2 [512,256] x64 -> [128, (u, s, n)] fp8
    wf28 = fp8(
        (64.0 * Wf2).reshape(2, 2, 128, D2).transpose(2, 0, 1, 3).reshape(128, 1024)
    )
    vpack = np.concatenate(
        [32.0 * bqp.reshape(1, D), 32.0 * bkp.reshape(1, D), bf1p.reshape(4, D)],
        axis=0,
    ).T.astype(np.float32)
    shared = {
        "wpack": np.ascontiguousarray(wpack),
        "wf18": np.ascontiguousarray(wf18),
        "wf28": np.ascontiguousarray(wf28),
        "vpack": np.ascontiguousarray(vpack),
        "bf2": f("bf2"), "bo": bop,
        "zpad": np.zeros((128, L), dtype=ml_dtypes.float8_e4m3),
    }

    in_maps = []
    for c in range(8):
        b, h = c // 2, c % 2
        if h == 0:
            x1c, x2c = x1[b], x2[b]
        else:
            x1c = np.concatenate([x1[b, A:], x1[b, :A]], axis=0)
            x2c = np.concatenate([x2[b, A:], x2[b, :A]], axis=0)
        tilep = lambda M, nt: np.ascontiguousarray(
            M.reshape(nt, 128, D).transpose(1, 0, 2)
        )
        m = dict(shared)
        m["x1"] = tilep(x1c, NT)
        m["x2"] = tilep(x2c, NT)
        m["res1p"] = tilep(x1c[:A] + bp1p, AT)
        m["res2p"] = tilep(x2c[:A] + bp2p, AT)
        in_maps.append(m)

    nc = _get_nc()
    res = run_bass_kernel_spmd(nc, in_maps, core_ids=list(range(8)))
    out = np.empty((B, L, OUT), np.float32)
    for c in range(8):
        b, h = c // 2, c % 2
        oc = res.results[c]["out"].transpose(1, 0, 2).reshape(A, OUT)
        out[b, h * A : (h + 1) * A, :] = oc
    return out



# revision 69
# speedup vs baseline: 1.0033x; 1.0033x over previous
"""Trainium2 Bass kernel for the dual-stream encoder block.

Sharding: 8 cores = 4 batches x 2 query-row halves (2048 rows/core).
Inputs are pre-rolled along L per core so output rows are always 0..2047;
K/V contraction uses the full 4096 rows. No cross-core communication.
"""

import sys

sys.path.insert(0, "/opt/trn_rl_repo")

import numpy as np
import ml_dtypes

B, L, D, OUT = 4, 4096, 128, 55
D2, H = 256, 512
A = 2048  # output rows per core
NT = 32  # l-tiles of 128
NG = 8  # l-groups of 4 tiles
AT = 16  # a-tiles per core
AC = 4  # a-chunks of 512
SCALE = float(1.0 / np.sqrt(np.float32(128.0)))

_CACHE = {}


def _build_nc():
    import concourse.bass as bass
    from concourse import bacc, mybir
    import concourse.tile as tile
    from concourse.masks import make_identity

    f32 = mybir.dt.float32
    bf16 = mybir.dt.bfloat16
    AF = mybir.ActivationFunctionType
    ALU = mybir.AluOpType

    fp8 = mybir.dt.float8e4

    nc = bacc.Bacc("TRN2", target_bir_lowering=False, debug=False)

    # ---- DRAM parameters -------------------------------------------------
    dx1 = nc.dram_tensor("x1", [128, NT, D], f32, kind="ExternalInput")
    dx2 = nc.dram_tensor("x2", [128, NT, D], f32, kind="ExternalInput")
    dres1 = nc.dram_tensor("res1p", [128, AT, D], f32, kind="ExternalInput")
    dres2 = nc.dram_tensor("res2p", [128, AT, D], f32, kind="ExternalInput")
    dwpack = nc.dram_tensor("wpack", [128, 878], bf16, kind="ExternalInput")
    dwf18 = nc.dram_tensor("wf18", [128, 1024], fp8, kind="ExternalInput")
    dwf28 = nc.dram_tensor("wf28", [128, 1024], fp8, kind="ExternalInput")
    dvpack = nc.dram_tensor("vpack", [128, 6], f32, kind="ExternalInput")
    dbf2 = nc.dram_tensor("bf2", [D2], f32, kind="ExternalInput")
    dbo = nc.dram_tensor("bo", [OUT], f32, kind="ExternalInput")
    dzpad = nc.dram_tensor("zpad", [128, L], fp8, kind="ExternalInput")
    dout = nc.dram_tensor("out", [128, AT, OUT], f32, kind="ExternalOutput")

    def bcast_ap(dt_handle, n):
        ap = dt_handle.ap()
        return bass.AP(tensor=ap.tensor, offset=ap.offset, ap=[[0, 128], [1, n]])

    with tile.TileContext(nc) as tc:
        import contextlib

        with contextlib.ExitStack() as ctx:
            consts = ctx.enter_context(tc.tile_pool(name="consts", bufs=1))
            big = ctx.enter_context(tc.tile_pool(name="big", bufs=1))
            stats = ctx.enter_context(tc.tile_pool(name="stats", bufs=3))
            xpool = ctx.enter_context(tc.tile_pool(name="xpool", bufs=4))
            xspool = ctx.enter_context(tc.tile_pool(name="xs", bufs=5))
            respool = ctx.enter_context(tc.tile_pool(name="res", bufs=1))
            outpool = ctx.enter_context(tc.tile_pool(name="outp", bufs=1))

            DR = mybir.MatmulPerfMode.DoubleRow

            # ---- constants ----
            ident = consts.tile([128, 128], bf16)
            make_identity(nc, ident[:])
            ones8 = consts.tile([128, 2, 1], fp8)
            nc.vector.memset(ones8[:], 1.0)

            wpk = consts.tile([128, 878], bf16)
            wf18 = consts.tile([128, 2, 4, 128], fp8)
            wf28 = consts.tile([128, 2, 2, 256], fp8)
            vpk = consts.tile([128, 6], f32)
            wq = wpk[:, 0:128]
            wk = wpk[:, 128:256]
            wv1 = wpk[:, 256:384]
            wv2 = wpk[:, 384:512]
            wp1 = wpk[:, 512:640]
            wp2 = wpk[:, 640:768]
            wov = lambda sh: wpk[:, 768 + 55 * sh : 768 + 55 * (sh + 1)]
            bq = vpk[:, 0:1]
            bk = vpk[:, 1:2]
            bf1t = vpk[:, 2:6]
            bf2b = consts.tile([128, D2], f32)
            bob = consts.tile([128, OUT], f32)

            # ---- big SBUF residents ----
            x1nT = big.tile([128, L], bf16)  # [d, l]
            x2nT = big.tile([128, L], bf16)
            q1T8 = big.tile([128, 2, L], fp8)  # [d, (real|zero), l] x32
            k2T8 = big.tile([128, 2, A], fp8)
            v1_8 = big.tile([128, NT, 128], fp8)  # [l-part, tile, d] x32
            v2_8 = big.tile([128, NT, 128], fp8)
            attF = [
                big.tile([128, NT, 512], fp8, name="attF0", tag="attF0"),
                big.tile([128, NT, 512], fp8, name="attF1", tag="attF1"),
            ]
            o1T = big.tile([128, A], bf16)  # (att @ v1)^T, unnorm, /32
            o2T = big.tile([128, A], bf16)
            invd = big.tile([128, AT], f32)
            xcat = big.tile([128, AT, D2], f32)
            xfT8 = big.tile([128, 2, A], fp8)  # lnf normalized^T x16
            x3Tl = big.tile([128, A], bf16)  # ln3 normalized^T halves
            x3Th = big.tile([128, A], bf16)
            h1T8 = big.tile([128, 4, A], fp8)

            # =========== Phase A (LN+transpose+QKV) interleaved with =====
            # =========== chunk-0 attention ===============================
            psB_cm = tc.tile_pool(name="psB", bufs=2, space="PSUM")
            psB = psB_cm.__enter__()
            psBo_cm = tc.tile_pool(name="psBo", bufs=1, space="PSUM")
            psBo = psBo_cm.__enter__()
            psA_cm = tc.tile_pool(name="psA", bufs=2, space="PSUM")
            psA = psA_cm.__enter__()

            BSs = [stats.tile([128, NT, 6], f32, tag="BSs%d" % s, name="BSs%d" % s) for s in (0, 1)]
            MVs = [stats.tile([128, NT, 2], f32, tag="MVs%d" % s, name="MVs%d" % s) for s in (0, 1)]
            RSs = [stats.tile([128, NT], f32, tag="RSs%d" % s, name="RSs%d" % s) for s in (0, 1)]
            nAs = [stats.tile([128, NT], f32, tag="nAs%d" % s, name="nAs%d" % s) for s in (0, 1)]
            nBs = [stats.tile([128, NT], f32, tag="nBs%d" % s, name="nBs%d" % s) for s in (0, 1)]
            BSf = stats.tile([128, AT, 6], f32, tag="BSf")
            MVf = stats.tile([128, AT, 2], f32, tag="MVf")
            RSf = stats.tile([128, AT], f32, tag="RSf")
            nt1 = stats.tile([128, AT], f32, tag="nt1")
            nt2 = stats.tile([128, AT], f32, tag="nt2")

            def newton_rsqrt(dst, var_ap, t1, t2, scale16=False, iters=3):
                # dst = rsqrt(var) [x16] on Pool; 1 + `iters` Newton steps,
                # var in ~[0.5, 2]; the x16 rides the last iteration free.
                nc.gpsimd.tensor_scalar(
                    dst, var_ap, -0.5, 1.5, op0=ALU.mult, op1=ALU.add
                )
                for it in range(iters):
                    last = it == iters - 1
                    c0, c1 = (-8.0, 24.0) if (last and scale16) else (-0.5, 1.5)
                    nc.gpsimd.tensor_tensor(t1, dst, dst, op=ALU.mult)
                    nc.gpsimd.tensor_tensor(t2, var_ap, t1, op=ALU.mult)
                    nc.gpsimd.tensor_scalar(
                        t1, t2, c0, c1, op0=ALU.mult, op1=ALU.add
                    )
                    nc.gpsimd.tensor_tensor(dst, dst, t1, op=ALU.mult)

            def ln_to_T(MVx, RSx, chunks, norm_eng, evac_act=False,
                        pool=None, tag="tr2"):
                # xcat rows -> x16-normalized fp8 transposed halves in xfT8
                for jj in chunks:
                    psT2 = (pool or psC).tile([128, 4, 256], bf16, tag=tag)
                    for k in range(4):
                        t = 4 * jj + k
                        xsf = xspool.tile([128, D2], bf16, tag="xsf")
                        norm_eng.tensor_scalar(
                            xsf[:],
                            xcat[:, t, :],
                            MVx[:, t, 0:1],
                            RSx[:, t : t + 1],
                            op0=ALU.subtract,
                            op1=ALU.mult,
                        )
                        nc.tensor.transpose(psT2[:, k, 0:128], xsf[:, 0:128], ident[:])
                        nc.tensor.transpose(psT2[:, k, 128:256], xsf[:, 128:256], ident[:])
                    if evac_act:
                        nc.scalar.activation(
                            xfT8[:, 0, 512 * jj : 512 * (jj + 1)],
                            psT2[:, :, 0:128], AF.Identity,
                        )
                        nc.scalar.activation(
                            xfT8[:, 1, 512 * jj : 512 * (jj + 1)],
                            psT2[:, :, 128:256], AF.Identity,
                        )
                    else:
                        nc.vector.tensor_copy(
                            xfT8[:, 0, 512 * jj : 512 * (jj + 1)], psT2[:, :, 0:128]
                        )
                        nc.vector.tensor_copy(
                            xfT8[:, 1, 512 * jj : 512 * (jj + 1)], psT2[:, :, 128:256]
                        )


            Xhold = {}
            dxs = (dx1, dx2)
            xnTs = (x1nT, x2nT)

            def s_stats(s, g):
                X = xpool.tile([128, 4, 128], f32, tag="X%d" % s)
                nc.sync.dma_start(X[:], dxs[s].ap()[:, 4 * g : 4 * g + 4, :])
                for k in range(4):
                    i = 4 * g + k
                    nc.vector.bn_stats(BSs[s][:, i, :], X[:, k, :])
                    nc.vector.bn_aggr(MVs[s][:, i, :], BSs[s][:, i, :])
                Xhold[(s, g)] = X

            def s_xform(s, g):
                X = Xhold.pop((s, g))
                for k in range(4):
                    i = 4 * g + k
                    xs = xspool.tile([128, 128], bf16, tag="xs")
                    nc.gpsimd.tensor_scalar(
                        xs[:], X[:, k, :], MVs[s][:, i, 0:1], RSs[s][:, i : i + 1],
                        op0=ALU.subtract, op1=ALU.mult,
                    )
                    if k == 0:
                        psT = psA.tile([128, 4, 128], bf16, tag="ps")
                    nc.tensor.transpose(psT[:, k, :], xs[:], ident[:])
                nc.vector.tensor_copy(
                    xnTs[s][:, 512 * g : 512 * (g + 1)], psT[:]
                )

            def qkv1(g):
                psQ = psA.tile([128, 512], f32, tag="ps")
                nc.tensor.matmul(
                    psQ[:], wq, x1nT[:, 512 * g : 512 * (g + 1)],
                    start=True, stop=True,
                )
                nc.vector.tensor_scalar(
                    q1T8[:, 0, 512 * g : 512 * (g + 1)], psQ[:], 32.0, bq,
                    op0=ALU.mult, op1=ALU.add,
                )
                psV = psA.tile([128, 512], f32, tag="ps")
                for k in range(4):
                    i = 4 * g + k
                    nc.tensor.matmul(
                        psV[:, 128 * k : 128 * (k + 1)],
                        x1nT[:, 128 * i : 128 * (i + 1)], wv1,
                        start=True, stop=True,
                    )
                nc.vector.tensor_scalar(
                    v1_8[:, 4 * g : 4 * g + 4, :], psV[:], 32.0, None, op0=ALU.mult
                )

            def qkv2(g):
                if g < 4:
                    psK = psA.tile([128, 512], f32, tag="ps")
                    nc.tensor.matmul(
                        psK[:], wk, x2nT[:, 512 * g : 512 * (g + 1)],
                        start=True, stop=True,
                    )
                    nc.vector.tensor_scalar(
                        k2T8[:, 0, 512 * g : 512 * (g + 1)], psK[:], 32.0, bk,
                        op0=ALU.mult, op1=ALU.add,
                    )
                psV = psA.tile([128, 512], f32, tag="ps")
                for k in range(4):
                    i = 4 * g + k
                    nc.tensor.matmul(
                        psV[:, 128 * k : 128 * (k + 1)],
                        x2nT[:, 128 * i : 128 * (i + 1)], wv2,
                        start=True, stop=True,
                    )
                nc.scalar.activation(
                    v2_8[:, 4 * g : 4 * g + 4, :], psV[:], AF.Identity, scale=32.0
                )

            class Chunk:
                def __init__(self, j):
                    self.j = j
                    self.psO1 = psBo.tile([128, 512], f32, tag="o1")
                    self.psO2 = psBo.tile([128, 512], f32, tag="o2")
                    self.aF = attF[j % 2]

                def att_pair(self, g, sl=slice(None)):
                    return self.aF[:, 2 * g : 2 * g + 2, sl]

                def ex_pair(self, g):
                    psE = psB.tile([128, 2, 512], f32, tag="e")
                    for k in range(2):
                        i = 2 * g + k
                        nc.tensor.matmul(
                            psE[:, k, :],
                            q1T8[:, :, 128 * i : 128 * (i + 1)],
                            k2T8[:, :, 512 * self.j : 512 * (self.j + 1)],
                            start=True, stop=True, perf_mode=DR,
                        )
                    nc.scalar.activation(
                        self.att_pair(g), psE[:], AF.Exp, scale=SCALE / 1024.0
                    )

                def av_pair(self, g):
                    ap_ = self.att_pair(g)
                    last = g == NT // 2 - 1
                    nc.tensor.matmul(
                        self.psO1[:], v1_8[:, 2 * g : 2 * g + 2, :], ap_,
                        start=(g == 0), stop=last,
                        perf_mode=DR, skip_group_check=True,
                    )
                    nc.tensor.matmul(
                        self.psO2[:], v2_8[:, 2 * g : 2 * g + 2, :], ap_,
                        start=(g == 0), stop=last,
                        perf_mode=DR, skip_group_check=True,
                    )

                def den_pair(self, g, psDen):
                    last = g == NT // 2 - 1
                    for t in range(4):
                        nc.tensor.matmul(
                            psDen[:, t : t + 1],
                            self.att_pair(g, slice(128 * t, 128 * (t + 1))),
                            ones8[:],
                            start=(g == 0), stop=last,
                            perf_mode=DR, skip_group_check=True,
                        )

            # ---- interleaved head: 4 rounds x (2 groups/stream + pairs) ---
            # chunk-0 pairs ride each round; chunk-1 exps lag one round
            c0 = Chunk(0)
            c1 = Chunk(1)
            for m in range(4):
                for s in (0, 1):
                    s_stats(s, 2 * m)
                    s_stats(s, 2 * m + 1)
                if m == 0:
                    # weights + zero-pad halves load behind the first X DMAs
                    nc.scalar.dma_start(wpk[:], dwpack[:])
                    nc.scalar.dma_start(vpk[:], dvpack[:])
                    nc.sync.dma_start(q1T8[:, 1, :], dzpad[:])
                    nc.sync.dma_start(k2T8[:, 1, :], dzpad.ap()[:, 0:A])
                if m == 0:
                    for s in (0, 1):
                        for gg in (0, 1):
                            sl4 = slice(4 * gg, 4 * gg + 4)
                            newton_rsqrt(
                                RSs[s][:, sl4], MVs[s][:, sl4, 1],
                                nAs[s][:, sl4], nBs[s][:, sl4], iters=2,
                            )
                else:
                    sl8 = slice(8 * m, 8 * m + 8)
                    for s in (0, 1):
                        newton_rsqrt(
                            RSs[s][:, sl8], MVs[s][:, sl8, 1],
                            nAs[s][:, sl8], nBs[s][:, sl8], iters=2,
                        )
                for s in (0, 1):
                    s_xform(s, 2 * m)
                    s_xform(s, 2 * m + 1)
                qkv1(2 * m)
                qkv2(2 * m)
                qkv1(2 * m + 1)
                qkv2(2 * m + 1)
                for p in range(4 * m, 4 * m + 4):
                    c0.ex_pair(p)
                    if p > 0:
                        c0.av_pair(p - 1)
                    if m > 0:
                        c1.ex_pair(p - 4)
            c0.av_pair(NT // 2 - 1)
            for p in range(12, NT // 2):
                c1.ex_pair(p)

            psA_cm.__exit__(None, None, None)
            psD_cm = tc.tile_pool(name="psD", bufs=1, space="PSUM")
            psD = psD_cm.__enter__()
            psDen = psD.tile([128, AC, 4], f32, tag="den", name="psDen")

            r1 = respool.tile([128, AT, D], f32, tag="r1")
            r2 = respool.tile([128, AT, D], f32, tag="r2")
            r_loaded = set()

            def load_res(j):
                if j in r_loaded:
                    return
                r_loaded.add(j)
                nc.sync.dma_start(
                    r1[:, 4 * j : 4 * j + 4, :], dres1.ap()[:, 4 * j : 4 * j + 4, :]
                )
                nc.sync.dma_start(
                    r2[:, 4 * j : 4 * j + 4, :], dres2.ap()[:, 4 * j : 4 * j + 4, :]
                )

            def evac_chunk(c):
                j = c.j
                nc.vector.reciprocal(invd[:, 4 * j : 4 * j + 4], psDen[:, j, :])
                nc.vector.tensor_scalar(
                    o1T[:, 512 * j : 512 * (j + 1)], c.psO1[:], 1.0 / 32.0, None,
                    op0=ALU.mult,
                )
                nc.vector.tensor_scalar(
                    o2T[:, 512 * j : 512 * (j + 1)], c.psO2[:], 1.0 / 32.0, None,
                    op0=ALU.mult,
                )

            def proj_chunk(j):
                # output projection + residual + lnf stats for this chunk
                for k in range(4):
                    t = 4 * j + k
                    for (oT, wp, rr, off) in (
                        (o1T, wp1, r1[:, t, :], 0),
                        (o2T, wp2, r2[:, t, :], D),
                    ):
                        psP = psD.tile([128, D], f32, tag="p")
                        nc.tensor.matmul(
                            psP[:], oT[:, 128 * t : 128 * (t + 1)], wp,
                            start=True, stop=True,
                        )
                        sc = xspool.tile([128, D], f32, tag="sc")
                        nc.vector.tensor_scalar(
                            sc[:], psP[:], invd[:, t : t + 1], None, op0=ALU.mult
                        )
                        nc.gpsimd.tensor_tensor(
                            xcat[:, t, off : off + D], sc[:], rr, op=ALU.add
                        )
                    nc.vector.bn_stats(BSf[:, t, :], xcat[:, t, :])
                    nc.vector.bn_aggr(MVf[:, t, :], BSf[:, t, :])
                sl = slice(4 * j, 4 * (j + 1))
                newton_rsqrt(
                    RSf[:, sl], MVf[:, sl, 1], nt1[:, sl], nt2[:, sl], scale16=True
                )

            for g in range(NT // 2):
                c0.den_pair(g, psDen[:, 0, :])
            evac_chunk(c0)
            for g in range(NT // 2):
                c1.av_pair(g)
                c1.den_pair(g, psDen[:, 1, :])
            evac_chunk(c1)

            pend = [0, 1]
            for j in range(2, AC):
                c = Chunk(j)
                if j == 2:
                    # deferred FFN constants (transfer hides under attention)
                    nc.scalar.dma_start(wf18[:], dwf18[:])
                    nc.scalar.dma_start(wf28[:], dwf28[:])
                    nc.scalar.dma_start(bf2b[:], bcast_ap(dbf2, D2))
                    nc.scalar.dma_start(bob[:], bcast_ap(dbo, OUT))
                load_res(pend[0])
                if len(pend) > 1:
                    load_res(pend[1])
                for g in range(NT // 2):
                    c.ex_pair(g)
                    if g > 0:
                        c.av_pair(g - 1)
                        c.den_pair(g - 1, psDen[:, j, :])
                    if g in (5, 10) and pend:
                        proj_chunk(pend.pop(0))
                    if j == AC - 1 and g == 12:
                        # hoist lnf transpose of finished chunks under the
                        # last chunk's exp stream (psP bank is idle here)
                        ln_to_T(MVf, RSf, [0, 1, 2], norm_eng=nc.gpsimd,
                                pool=psD, tag="p")
                        for t in range(12):
                            nc.gpsimd.tensor_tensor(
                                xcat[:, t, :], xcat[:, t, :], bf2b[:],
                                op=ALU.add,
                            )
                c.av_pair(NT // 2 - 1)
                c.den_pair(NT // 2 - 1, psDen[:, j, :])
                evac_chunk(c)
                pend.append(j)
            # ---- FFN + outputs ----------------------------------------
            BS3 = stats.tile([128, AT, 6], f32, tag="BS3")
            MV3 = stats.tile([128, AT, 2], f32, tag="MV3")
            RS3 = stats.tile([128, AT], f32, tag="RS3")
            ov = dout.ap()
            osb = outpool.tile([128, AT, OUT], f32, tag="osb")

            def f1gelu(jj, pool, tag):
                for n in range(4):
                    psH = pool.tile([128, 512], f32, tag=tag, name="psH")
                    nc.tensor.matmul(
                        psH[:],
                        wf18[:, :, n, :],
                        xfT8[:, :, 512 * jj : 512 * (jj + 1)],
                        start=True,
                        stop=True,
                        perf_mode=DR,
                    )
                    nc.scalar.activation(
                        h1T8[:, n, 512 * jj : 512 * (jj + 1)],
                        psH[:],
                        AF.Gelu,
                        bias=bf1t[:, n : n + 1],
                        scale=1.0 / 1024.0,
                    )

            # gelus for hoisted chunks start right after the last exp,
            # scavenging the drained psE ring
            f1gelu(0, psB, "e")
            f1gelu(1, psB, "e")
            f1gelu(2, psB, "e")
            for j in pend:
                load_res(j)
            for j in pend:
                proj_chunk(j)
            ln_to_T(MVf, RSf, [3], norm_eng=nc.gpsimd, pool=psD, tag="p")
            for t in range(12, AT):
                nc.gpsimd.tensor_tensor(
                    xcat[:, t, :], xcat[:, t, :], bf2b[:], op=ALU.add
                )
            f1gelu(3, psB, "e")

            psD_cm.__exit__(None, None, None)
            psBo_cm.__exit__(None, None, None)
            psB_cm.__exit__(None, None, None)

            psC = ctx.enter_context(tc.tile_pool(name="psC", bufs=2, space="PSUM"))

            def out_chunk(jj):
                # ln3-normalize chunk jj (bf16) and project to the output
                psT2 = psC.tile([128, 4, 256], bf16, tag="tr2", name="psT3")
                for k in range(4):
                    t = 4 * jj + k
                    xsf = xspool.tile([128, D2], bf16, tag="xsf")
                    nc.gpsimd.tensor_scalar(
                        xsf[:], xcat[:, t, :], MV3[:, t, 0:1], RS3[:, t : t + 1],
                        op0=ALU.subtract, op1=ALU.mult,
                    )
                    nc.tensor.transpose(psT2[:, k, 0:128], xsf[:, 0:128], ident[:])
                    nc.tensor.transpose(psT2[:, k, 128:256], xsf[:, 128:256], ident[:])
                nc.scalar.activation(
                    x3Tl[:, 512 * jj : 512 * (jj + 1)], psT2[:, :, 0:128],
                    AF.Identity,
                )
                nc.vector.tensor_copy(
                    x3Th[:, 512 * jj : 512 * (jj + 1)], psT2[:, :, 128:256]
                )
                for t in range(4 * jj, 4 * jj + 4):
                    psO = psC.tile([128, OUT], f32, tag="po", name="psOu")
                    nc.tensor.matmul(
                        psO[:], x3Tl[:, 128 * t : 128 * (t + 1)], wov(0),
                        start=True, stop=False, skip_group_check=True,
                    )
                    nc.tensor.matmul(
                        psO[:], x3Th[:, 128 * t : 128 * (t + 1)], wov(1),
                        start=False, stop=True, skip_group_check=True,
                    )
                    nc.vector.tensor_tensor(osb[:, t, :], psO[:], bob[:], op=ALU.add)
                nc.sync.dma_start(
                    ov[:, 4 * jj : 4 * jj + 4, :], osb[:, 4 * jj : 4 * jj + 4, :]
                )

            for jj in range(AC):
                for t in range(4 * jj, 4 * jj + 4):
                    psH2 = psC.tile([128, D2], f32, tag="h2", name="psH2")
                    for u in range(2):
                        nc.tensor.matmul(
                            psH2[:],
                            h1T8[:, 2 * u : 2 * u + 2, 128 * t : 128 * (t + 1)],
                            wf28[:, u, :, :],
                            start=(u == 0),
                            stop=(u == 1),
                            perf_mode=DR,
                            skip_group_check=True,
                        )
                    nc.vector.scalar_tensor_tensor(
                        xcat[:, t, :], psH2[:], 1.0 / 64.0, xcat[:, t, :],
                        op0=ALU.mult, op1=ALU.add,
                    )
                    nc.vector.bn_stats(BS3[:, t, :], xcat[:, t, :])
                    nc.vector.bn_aggr(MV3[:, t, :], BS3[:, t, :])
                sl = slice(4 * jj, 4 * jj + 4)
                newton_rsqrt(
                    RS3[:, sl], MV3[:, sl, 1], nt1[:, sl], nt2[:, sl], iters=2
                )
                if jj >= 1:
                    out_chunk(jj - 1)
            out_chunk(3)

    nc.compile()
    return nc


def _get_nc():
    if "nc" not in _CACHE:
        _CACHE["nc"] = _build_nc()
    return _CACHE["nc"]


def kernel(**inputs):
    from concourse.bass_utils import run_bass_kernel_spmd

    f = lambda k: np.asarray(inputs[k], dtype=np.float32)
    bf = lambda a: np.asarray(a, dtype=np.float32).astype(ml_dtypes.bfloat16)

    x1, x2 = f("x1"), f("x2")
    g1, b1 = f("ln1_g"), f("ln1_b")
    g2, b2 = f("ln2_g"), f("ln2_b")
    gf_, bf_ = f("lnf_g"), f("lnf_b")
    g3, b3 = f("ln3_g"), f("ln3_b")
    # fold LN gains/biases into the adjacent linear layers
    Wq = g1[:, None] * f("Wq"); bqp = b1 @ f("Wq") + f("bq")
    Wk = g2[:, None] * f("Wk"); bkp = b2 @ f("Wk") + f("bk")
    Wv1 = g1[:, None] * f("Wv1"); bv1p = b1 @ f("Wv1") + f("bv1")
    Wv2 = g2[:, None] * f("Wv2"); bv2p = b2 @ f("Wv2") + f("bv2")
    Wf1 = gf_[:, None] * f("Wf1"); bf1p = bf_ @ f("Wf1") + f("bf1")
    Wo = g3[:, None] * f("Wo"); bop = b3 @ f("Wo") + f("bo")
    Wp1, Wp2 = f("Wp1"), f("Wp2")
    bp1p = bv1p @ Wp1 + f("bp1")
    bp2p = bv2p @ Wp2 + f("bp2")

    Wf2 = f("Wf2")
    fp8 = lambda a: np.asarray(a, dtype=np.float32).astype(ml_dtypes.float8_e4m3)
    wpack = np.concatenate(
        [bf(Wq), bf(Wk), bf(Wv1), bf(Wv2), bf(Wp1), bf(Wp2),
         bf(Wo).reshape(2, 128, OUT).transpose(1, 0, 2).reshape(128, 2 * OUT)],
        axis=1,
    )
    # Wf1 [256,512] x64 -> [128, (kh, n, np)] fp8
    wf18 = fp8(
        (64.0 * Wf1).reshape(2, 128, 4, 128).transpose(1, 0, 2, 3).reshape(128, 1024)
    )
    # Wf2 [512,256] x64 -> [128, (u, s, n)] fp8
    wf28 = fp8(
        (64.0 * Wf2).reshape(2, 2, 128, D2).transpose(2, 0, 1, 3).reshape(128, 1024)
    )
    vpack = np.concatenate(
        [32.0 * bqp.reshape(1, D), 32.0 * bkp.reshape(1, D), bf1p.reshape(4, D)],
        axis=0,
    ).T.astype(np.float32)
    shared = {
        "wpack": np.ascontiguousarray(wpack),
        "wf18": np.ascontiguousarray(wf18),
        "wf28": np.ascontiguousarray(wf28),
        "vpack": np.ascontiguousarray(vpack),
        "bf2": f("bf2"), "bo": bop,
        "zpad": np.zeros((128, L), dtype=ml_dtypes.float8_e4m3),
    }

    in_maps = []
    for c in range(8):
        b, h = c // 2, c % 2
        if h == 0:
            x1c, x2c = x1[b], x2[b]
        else:
            x1c = np.concatenate([x1[b, A:], x1[b, :A]], axis=0)
            x2c = np.concatenate([x2[b, A:], x2[b, :A]], axis=0)
        tilep = lambda M, nt: np.ascontiguousarray(
            M.reshape(nt, 128, D).transpose(1, 0, 2)
        )
        m = dict(shared)
        m["x1"] = tilep(x1c, NT)
        m["x2"] = tilep(x2c, NT)
        m["res1p"] = tilep(x1c[:A] + bp1p, AT)
        m["res2p"] = tilep(x2c[:A] + bp2p, AT)
        in_maps.append(m)

    nc = _get_nc()
    res = run_bass_kernel_spmd(nc, in_maps, core_ids=list(range(8)))
    out = np.empty((B, L, OUT), np.float32)
    for c in range(8):
        b, h = c // 2, c % 2
        oc = res.results[c]["out"].transpose(1, 0, 2).reshape(A, OUT)
        out[b, h * A : (h + 1) * A, :] = oc
    return out



# revision 70
# speedup vs baseline: 1.0051x; 1.0018x over previous
"""Trainium2 Bass kernel for the dual-stream encoder block.

Sharding: 8 cores = 4 batches x 2 query-row halves (2048 rows/core).
Inputs are pre-rolled along L per core so output rows are always 0..2047;
K/V contraction uses the full 4096 rows. No cross-core communication.
"""

import sys

sys.path.insert(0, "/opt/trn_rl_repo")

import numpy as np
import ml_dtypes

B, L, D, OUT = 4, 4096, 128, 55
D2, H = 256, 512
A = 2048  # output rows per core
NT = 32  # l-tiles of 128
NG = 8  # l-groups of 4 tiles
AT = 16  # a-tiles per core
AC = 4  # a-chunks of 512
SCALE = float(1.0 / np.sqrt(np.float32(128.0)))

_CACHE = {}


def _build_nc():
    import concourse.bass as bass
    from concourse import bacc, mybir
    import concourse.tile as tile
    from concourse.masks import make_identity

    f32 = mybir.dt.float32
    bf16 = mybir.dt.bfloat16
    AF = mybir.ActivationFunctionType
    ALU = mybir.AluOpType

    fp8 = mybir.dt.float8e4

    nc = bacc.Bacc("TRN2", target_bir_lowering=False, debug=False)

    # ---- DRAM parameters -------------------------------------------------
    dx1 = nc.dram_tensor("x1", [128, NT, D], f32, kind="ExternalInput")
    dx2 = nc.dram_tensor("x2", [128, NT, D], f32, kind="ExternalInput")
    dres1 = nc.dram_tensor("res1p", [128, AT, D], f32, kind="ExternalInput")
    dres2 = nc.dram_tensor("res2p", [128, AT, D], f32, kind="ExternalInput")
    dwpack = nc.dram_tensor("wpack", [128, 878], bf16, kind="ExternalInput")
    dwf18 = nc.dram_tensor("wf18", [128, 1024], fp8, kind="ExternalInput")
    dwf28 = nc.dram_tensor("wf28", [128, 1024], fp8, kind="ExternalInput")
    dvpack = nc.dram_tensor("vpack", [128, 6], f32, kind="ExternalInput")
    dbf2 = nc.dram_tensor("bf2", [D2], f32, kind="ExternalInput")
    dbo = nc.dram_tensor("bo", [OUT], f32, kind="ExternalInput")
    dzpad = nc.dram_tensor("zpad", [128, L], fp8, kind="ExternalInput")
    dout = nc.dram_tensor("out", [128, AT, OUT], f32, kind="ExternalOutput")

    def bcast_ap(dt_handle, n):
        ap = dt_handle.ap()
        return bass.AP(tensor=ap.tensor, offset=ap.offset, ap=[[0, 128], [1, n]])

    with tile.TileContext(nc) as tc:
        import contextlib

        with contextlib.ExitStack() as ctx:
            consts = ctx.enter_context(tc.tile_pool(name="consts", bufs=1))
            big = ctx.enter_context(tc.tile_pool(name="big", bufs=1))
            stats = ctx.enter_context(tc.tile_pool(name="stats", bufs=3))
            xpool = ctx.enter_context(tc.tile_pool(name="xpool", bufs=4))
            xspool = ctx.enter_context(tc.tile_pool(name="xs", bufs=5))
            respool = ctx.enter_context(tc.tile_pool(name="res", bufs=1))
            outpool = ctx.enter_context(tc.tile_pool(name="outp", bufs=1))

            DR = mybir.MatmulPerfMode.DoubleRow

            # ---- constants ----
            ident = consts.tile([128, 128], bf16)
            make_identity(nc, ident[:])
            ones8 = consts.tile([128, 2, 1], fp8)
            nc.vector.memset(ones8[:], 1.0)

            wpk = consts.tile([128, 878], bf16)
            wf18 = consts.tile([128, 2, 4, 128], fp8)
            wf28 = consts.tile([128, 2, 2, 256], fp8)
            vpk = consts.tile([128, 6], f32)
            wq = wpk[:, 0:128]
            wk = wpk[:, 128:256]
            wv1 = wpk[:, 256:384]
            wv2 = wpk[:, 384:512]
            wp1 = wpk[:, 512:640]
            wp2 = wpk[:, 640:768]
            wov = lambda sh: wpk[:, 768 + 55 * sh : 768 + 55 * (sh + 1)]
            bq = vpk[:, 0:1]
            bk = vpk[:, 1:2]
            bf1t = vpk[:, 2:6]
            bf2b = consts.tile([128, D2], f32)
            bob = consts.tile([128, OUT], f32)

            # ---- big SBUF residents ----
            x1nT = big.tile([128, L], bf16)  # [d, l]
            x2nT = big.tile([128, L], bf16)
            q1T8 = big.tile([128, 2, L], fp8)  # [d, (real|zero), l] x32
            k2T8 = big.tile([128, 2, A], fp8)
            v1_8 = big.tile([128, NT, 128], fp8)  # [l-part, tile, d] x32
            v2_8 = big.tile([128, NT, 128], fp8)
            attF = [
                big.tile([128, NT, 512], fp8, name="attF0", tag="attF0"),
                big.tile([128, NT, 512], fp8, name="attF1", tag="attF1"),
            ]
            o1T = big.tile([128, A], bf16)  # (att @ v1)^T, unnorm, /32
            o2T = big.tile([128, A], bf16)
            invd = big.tile([128, AT], f32)
            xcat = big.tile([128, AT, D2], f32)
            xfT8 = big.tile([128, 2, A], fp8)  # lnf normalized^T x16
            x3Tl = big.tile([128, A], bf16)  # ln3 normalized^T halves
            x3Th = big.tile([128, A], bf16)
            h1T8 = big.tile([128, 4, A], fp8)

            # =========== Phase A (LN+transpose+QKV) interleaved with =====
            # =========== chunk-0 attention ===============================
            psB_cm = tc.tile_pool(name="psB", bufs=2, space="PSUM")
            psB = psB_cm.__enter__()
            psBo_cm = tc.tile_pool(name="psBo", bufs=1, space="PSUM")
            psBo = psBo_cm.__enter__()
            psA_cm = tc.tile_pool(name="psA", bufs=2, space="PSUM")
            psA = psA_cm.__enter__()

            BSs = [stats.tile([128, NT, 6], f32, tag="BSs%d" % s, name="BSs%d" % s) for s in (0, 1)]
            MVs = [stats.tile([128, NT, 2], f32, tag="MVs%d" % s, name="MVs%d" % s) for s in (0, 1)]
            RSs = [stats.tile([128, NT], f32, tag="RSs%d" % s, name="RSs%d" % s) for s in (0, 1)]
            nAs = [stats.tile([128, NT], f32, tag="nAs%d" % s, name="nAs%d" % s) for s in (0, 1)]
            nBs = [stats.tile([128, NT], f32, tag="nBs%d" % s, name="nBs%d" % s) for s in (0, 1)]
            BSf = stats.tile([128, AT, 6], f32, tag="BSf")
            MVf = stats.tile([128, AT, 2], f32, tag="MVf")
            RSf = stats.tile([128, AT], f32, tag="RSf")
            nt1 = stats.tile([128, AT], f32, tag="nt1")
            nt2 = stats.tile([128, AT], f32, tag="nt2")

            def newton_rsqrt(dst, var_ap, t1, t2, scale16=False, iters=3):
                # dst = rsqrt(var) [x16] on Pool; 1 + `iters` Newton steps,
                # var in ~[0.5, 2]; the x16 rides the last iteration free.
                nc.gpsimd.tensor_scalar(
                    dst, var_ap, -0.5, 1.5, op0=ALU.mult, op1=ALU.add
                )
                for it in range(iters):
                    last = it == iters - 1
                    c0, c1 = (-8.0, 24.0) if (last and scale16) else (-0.5, 1.5)
                    nc.gpsimd.tensor_tensor(t1, dst, dst, op=ALU.mult)
                    nc.gpsimd.tensor_tensor(t2, var_ap, t1, op=ALU.mult)
                    nc.gpsimd.tensor_scalar(
                        t1, t2, c0, c1, op0=ALU.mult, op1=ALU.add
                    )
                    nc.gpsimd.tensor_tensor(dst, dst, t1, op=ALU.mult)

            def ln_to_T(MVx, RSx, chunks, norm_eng, evac_act=False,
                        pool=None, tag="tr2"):
                # xcat rows -> x16-normalized fp8 transposed halves in xfT8
                for jj in chunks:
                    psT2 = (pool or psC).tile([128, 4, 256], bf16, tag=tag)
                    for k in range(4):
                        t = 4 * jj + k
                        xsf = xspool.tile([128, D2], bf16, tag="xsf")
                        norm_eng.tensor_scalar(
                            xsf[:],
                            xcat[:, t, :],
                            MVx[:, t, 0:1],
                            RSx[:, t : t + 1],
                            op0=ALU.subtract,
                            op1=ALU.mult,
                        )
                        nc.tensor.transpose(psT2[:, k, 0:128], xsf[:, 0:128], ident[:])
                        nc.tensor.transpose(psT2[:, k, 128:256], xsf[:, 128:256], ident[:])
                    if evac_act:
                        nc.scalar.activation(
                            xfT8[:, 0, 512 * jj : 512 * (jj + 1)],
                            psT2[:, :, 0:128], AF.Identity,
                        )
                        nc.scalar.activation(
                            xfT8[:, 1, 512 * jj : 512 * (jj + 1)],
                            psT2[:, :, 128:256], AF.Identity,
                        )
                    else:
                        nc.vector.tensor_copy(
                            xfT8[:, 0, 512 * jj : 512 * (jj + 1)], psT2[:, :, 0:128]
                        )
                        nc.vector.tensor_copy(
                            xfT8[:, 1, 512 * jj : 512 * (jj + 1)], psT2[:, :, 128:256]
                        )


            Xhold = {}
            dxs = (dx1, dx2)
            xnTs = (x1nT, x2nT)

            def s_stats(s, g):
                X = xpool.tile([128, 4, 128], f32, tag="X%d" % s)
                nc.sync.dma_start(X[:], dxs[s].ap()[:, 4 * g : 4 * g + 4, :])
                for k in range(4):
                    i = 4 * g + k
                    nc.vector.bn_stats(BSs[s][:, i, :], X[:, k, :])
                    nc.vector.bn_aggr(MVs[s][:, i, :], BSs[s][:, i, :])
                Xhold[(s, g)] = X

            def s_xform(s, g):
                X = Xhold.pop((s, g))
                for k in range(4):
                    i = 4 * g + k
                    xs = xspool.tile([128, 128], bf16, tag="xs")
                    nc.gpsimd.tensor_scalar(
                        xs[:], X[:, k, :], MVs[s][:, i, 0:1], RSs[s][:, i : i + 1],
                        op0=ALU.subtract, op1=ALU.mult,
                    )
                    if k == 0:
                        psT = psA.tile([128, 4, 128], bf16, tag="ps")
                    nc.tensor.transpose(psT[:, k, :], xs[:], ident[:])
                nc.vector.tensor_copy(
                    xnTs[s][:, 512 * g : 512 * (g + 1)], psT[:]
                )

            def qkv1(g):
                psQ = psA.tile([128, 512], f32, tag="ps")
                nc.tensor.matmul(
                    psQ[:], wq, x1nT[:, 512 * g : 512 * (g + 1)],
                    start=True, stop=True,
                )
                nc.vector.tensor_scalar(
                    q1T8[:, 0, 512 * g : 512 * (g + 1)], psQ[:], 32.0, bq,
                    op0=ALU.mult, op1=ALU.add,
                )
                psV = psA.tile([128, 512], f32, tag="ps")
                for k in range(4):
                    i = 4 * g + k
                    nc.tensor.matmul(
                        psV[:, 128 * k : 128 * (k + 1)],
                        x1nT[:, 128 * i : 128 * (i + 1)], wv1,
                        start=True, stop=True,
                    )
                nc.vector.tensor_scalar(
                    v1_8[:, 4 * g : 4 * g + 4, :], psV[:], 32.0, None, op0=ALU.mult
                )

            def qkv2(g):
                if g < 4:
                    psK = psA.tile([128, 512], f32, tag="ps")
                    nc.tensor.matmul(
                        psK[:], wk, x2nT[:, 512 * g : 512 * (g + 1)],
                        start=True, stop=True,
                    )
                    nc.vector.tensor_scalar(
                        k2T8[:, 0, 512 * g : 512 * (g + 1)], psK[:], 32.0, bk,
                        op0=ALU.mult, op1=ALU.add,
                    )
                psV = psA.tile([128, 512], f32, tag="ps")
                for k in range(4):
                    i = 4 * g + k
                    nc.tensor.matmul(
                        psV[:, 128 * k : 128 * (k + 1)],
                        x2nT[:, 128 * i : 128 * (i + 1)], wv2,
                        start=True, stop=True,
                    )
                nc.scalar.activation(
                    v2_8[:, 4 * g : 4 * g + 4, :], psV[:], AF.Identity, scale=32.0
                )

            class Chunk:
                def __init__(self, j):
                    self.j = j
                    self.psO1 = psBo.tile([128, 512], f32, tag="o1")
                    self.psO2 = psBo.tile([128, 512], f32, tag="o2")
                    self.aF = attF[j % 2]

                def att_pair(self, g, sl=slice(None)):
                    return self.aF[:, 2 * g : 2 * g + 2, sl]

                def ex_pair(self, g):
                    psE = psB.tile([128, 2, 512], f32, tag="e")
                    for k in range(2):
                        i = 2 * g + k
                        nc.tensor.matmul(
                            psE[:, k, :],
                            q1T8[:, :, 128 * i : 128 * (i + 1)],
                            k2T8[:, :, 512 * self.j : 512 * (self.j + 1)],
                            start=True, stop=True, perf_mode=DR,
                        )
                    nc.scalar.activation(
                        self.att_pair(g), psE[:], AF.Exp, scale=SCALE / 1024.0
                    )

                def av_pair(self, g):
                    ap_ = self.att_pair(g)
                    last = g == NT // 2 - 1
                    nc.tensor.matmul(
                        self.psO1[:], v1_8[:, 2 * g : 2 * g + 2, :], ap_,
                        start=(g == 0), stop=last,
                        perf_mode=DR, skip_group_check=True,
                    )
                    nc.tensor.matmul(
                        self.psO2[:], v2_8[:, 2 * g : 2 * g + 2, :], ap_,
                        start=(g == 0), stop=last,
                        perf_mode=DR, skip_group_check=True,
                    )

                def den_pair(self, g, psDen):
                    last = g == NT // 2 - 1
                    for t in range(4):
                        nc.tensor.matmul(
                            psDen[:, t : t + 1],
                            self.att_pair(g, slice(128 * t, 128 * (t + 1))),
                            ones8[:],
                            start=(g == 0), stop=last,
                            perf_mode=DR, skip_group_check=True,
                        )

            # ---- interleaved head: 4 rounds x (2 groups/stream + pairs) ---
            # chunk-0 pairs ride each round; chunk-1 exps lag one round
            c0 = Chunk(0)
            c1 = Chunk(1)
            for m in range(4):
                for s in (0, 1):
                    s_stats(s, 2 * m)
                    s_stats(s, 2 * m + 1)
                if m == 0:
                    # weights + zero-pad halves load behind the first X DMAs
                    nc.scalar.dma_start(wpk[:], dwpack[:])
                    nc.scalar.dma_start(vpk[:], dvpack[:])
                    nc.sync.dma_start(q1T8[:, 1, :], dzpad[:])
                    nc.sync.dma_start(k2T8[:, 1, :], dzpad.ap()[:, 0:A])
                if m == 0:
                    for s in (0, 1):
                        for gg in (0, 1):
                            sl4 = slice(4 * gg, 4 * gg + 4)
                            newton_rsqrt(
                                RSs[s][:, sl4], MVs[s][:, sl4, 1],
                                nAs[s][:, sl4], nBs[s][:, sl4], iters=2,
                            )
                else:
                    sl8 = slice(8 * m, 8 * m + 8)
                    for s in (0, 1):
                        newton_rsqrt(
                            RSs[s][:, sl8], MVs[s][:, sl8, 1],
                            nAs[s][:, sl8], nBs[s][:, sl8], iters=2,
                        )
                for s in (0, 1):
                    s_xform(s, 2 * m)
                    s_xform(s, 2 * m + 1)
                qkv1(2 * m)
                qkv2(2 * m)
                qkv1(2 * m + 1)
                qkv2(2 * m + 1)
                for p in range(4 * m, 4 * m + 4):
                    c0.ex_pair(p)
                    if p > 0:
                        c0.av_pair(p - 1)
                    if m > 0:
                        c1.ex_pair(p - 4)
            c0.av_pair(NT // 2 - 1)
            for p in range(12, NT // 2):
                c1.ex_pair(p)

            psA_cm.__exit__(None, None, None)
            psD_cm = tc.tile_pool(name="psD", bufs=1, space="PSUM")
            psD = psD_cm.__enter__()
            psDen = psD.tile([128, AC, 4], f32, tag="den", name="psDen")

            r1 = respool.tile([128, AT, D], f32, tag="r1")
            r2 = respool.tile([128, AT, D], f32, tag="r2")
            r_loaded = set()

            def load_res(j):
                if j in r_loaded:
                    return
                r_loaded.add(j)
                nc.sync.dma_start(
                    r1[:, 4 * j : 4 * j + 4, :], dres1.ap()[:, 4 * j : 4 * j + 4, :]
                )
                nc.sync.dma_start(
                    r2[:, 4 * j : 4 * j + 4, :], dres2.ap()[:, 4 * j : 4 * j + 4, :]
                )

            def evac_chunk(c):
                j = c.j
                nc.vector.reciprocal(invd[:, 4 * j : 4 * j + 4], psDen[:, j, :])
                nc.vector.tensor_scalar(
                    o1T[:, 512 * j : 512 * (j + 1)], c.psO1[:], 1.0 / 32.0, None,
                    op0=ALU.mult,
                )
                nc.vector.tensor_scalar(
                    o2T[:, 512 * j : 512 * (j + 1)], c.psO2[:], 1.0 / 32.0, None,
                    op0=ALU.mult,
                )

            def proj_chunk(j):
                # output projection + residual + lnf stats for this chunk
                for k in range(4):
                    t = 4 * j + k
                    for (oT, wp, rr, off) in (
                        (o1T, wp1, r1[:, t, :], 0),
                        (o2T, wp2, r2[:, t, :], D),
                    ):
                        psP = psD.tile([128, D], f32, tag="p")
                        nc.tensor.matmul(
                            psP[:], oT[:, 128 * t : 128 * (t + 1)], wp,
                            start=True, stop=True,
                        )
                        sc = xspool.tile([128, D], f32, tag="sc")
                        nc.vector.tensor_scalar(
                            sc[:], psP[:], invd[:, t : t + 1], None, op0=ALU.mult
                        )
                        nc.gpsimd.tensor_tensor(
                            xcat[:, t, off : off + D], sc[:], rr, op=ALU.add
                        )
                    nc.vector.bn_stats(BSf[:, t, :], xcat[:, t, :])
                    nc.vector.bn_aggr(MVf[:, t, :], BSf[:, t, :])
                sl = slice(4 * j, 4 * (j + 1))
                newton_rsqrt(
                    RSf[:, sl], MVf[:, sl, 1], nt1[:, sl], nt2[:, sl], scale16=True
                )

            for g in range(NT // 2):
                c0.den_pair(g, psDen[:, 0, :])
            evac_chunk(c0)
            for g in range(NT // 2):
                c1.av_pair(g)
                c1.den_pair(g, psDen[:, 1, :])
            evac_chunk(c1)

            pend = [0, 1]
            for j in range(2, AC):
                c = Chunk(j)
                if j == 2:
                    # deferred FFN constants (transfer hides under attention)
                    nc.scalar.dma_start(wf18[:], dwf18[:])
                    nc.scalar.dma_start(wf28[:], dwf28[:])
                    nc.scalar.dma_start(bf2b[:], bcast_ap(dbf2, D2))
                    nc.scalar.dma_start(bob[:], bcast_ap(dbo, OUT))
                load_res(pend[0])
                if len(pend) > 1:
                    load_res(pend[1])
                for g in range(NT // 2):
                    c.ex_pair(g)
                    if g > 0:
                        c.av_pair(g - 1)
                        c.den_pair(g - 1, psDen[:, j, :])
                    if g in (5, 10) and pend:
                        proj_chunk(pend.pop(0))
                    hoist = {(2, 8): 0, (3, 2): 1, (3, 8): 2}.get((j, g))
                    if hoist is not None:
                        # hoist lnf transpose of a finished chunk under the
                        # exp stream (psP bank + DVE/Pool slack)
                        ln_to_T(MVf, RSf, [hoist], norm_eng=nc.vector,
                                pool=psD, tag="p")
                        for t in range(4 * hoist, 4 * hoist + 4):
                            nc.gpsimd.tensor_tensor(
                                xcat[:, t, :], xcat[:, t, :], bf2b[:],
                                op=ALU.add,
                            )
                c.av_pair(NT // 2 - 1)
                c.den_pair(NT // 2 - 1, psDen[:, j, :])
                evac_chunk(c)
                pend.append(j)
            # ---- FFN + outputs ----------------------------------------
            BS3 = stats.tile([128, AT, 6], f32, tag="BS3")
            MV3 = stats.tile([128, AT, 2], f32, tag="MV3")
            RS3 = stats.tile([128, AT], f32, tag="RS3")
            ov = dout.ap()
            osb = outpool.tile([128, AT, OUT], f32, tag="osb")

            def f1gelu(jj, pool, tag):
                for n in range(4):
                    psH = pool.tile([128, 512], f32, tag=tag, name="psH")
                    nc.tensor.matmul(
                        psH[:],
                        wf18[:, :, n, :],
                        xfT8[:, :, 512 * jj : 512 * (jj + 1)],
                        start=True,
                        stop=True,
                        perf_mode=DR,
                    )
                    nc.scalar.activation(
                        h1T8[:, n, 512 * jj : 512 * (jj + 1)],
                        psH[:],
                        AF.Gelu,
                        bias=bf1t[:, n : n + 1],
                        scale=1.0 / 1024.0,
                    )

            # gelus for hoisted chunks start right after the last exp,
            # scavenging the drained psE ring
            f1gelu(0, psB, "e")
            f1gelu(1, psB, "e")
            f1gelu(2, psB, "e")
            for j in pend:
                load_res(j)
            for j in pend:
                proj_chunk(j)
            ln_to_T(MVf, RSf, [3], norm_eng=nc.gpsimd, pool=psD, tag="p")
            for t in range(12, AT):
                nc.gpsimd.tensor_tensor(
                    xcat[:, t, :], xcat[:, t, :], bf2b[:], op=ALU.add
                )
            f1gelu(3, psB, "e")

            psD_cm.__exit__(None, None, None)
            psBo_cm.__exit__(None, None, None)
            psB_cm.__exit__(None, None, None)

            psC = ctx.enter_context(tc.tile_pool(name="psC", bufs=2, space="PSUM"))

            def out_chunk(jj):
                # ln3-normalize chunk jj (bf16) and project to the output
                psT2 = psC.tile([128, 4, 256], bf16, tag="tr2", name="psT3")
                for k in range(4):
                    t = 4 * jj + k
                    xsf = xspool.tile([128, D2], bf16, tag="xsf")
                    nc.gpsimd.tensor_scalar(
                        xsf[:], xcat[:, t, :], MV3[:, t, 0:1], RS3[:, t : t + 1],
                        op0=ALU.subtract, op1=ALU.mult,
                    )
                    nc.tensor.transpose(psT2[:, k, 0:128], xsf[:, 0:128], ident[:])
                    nc.tensor.transpose(psT2[:, k, 128:256], xsf[:, 128:256], ident[:])
                nc.scalar.activation(
                    x3Tl[:, 512 * jj : 512 * (jj + 1)], psT2[:, :, 0:128],
                    AF.Identity,
                )
                nc.vector.tensor_copy(
                    x3Th[:, 512 * jj : 512 * (jj + 1)], psT2[:, :, 128:256]
                )
                for t in range(4 * jj, 4 * jj + 4):
                    psO = psC.tile([128, OUT], f32, tag="po", name="psOu")
                    nc.tensor.matmul(
                        psO[:], x3Tl[:, 128 * t : 128 * (t + 1)], wov(0),
                        start=True, stop=False, skip_group_check=True,
                    )
                    nc.tensor.matmul(
                        psO[:], x3Th[:, 128 * t : 128 * (t + 1)], wov(1),
                        start=False, stop=True, skip_group_check=True,
                    )
                    nc.vector.tensor_tensor(osb[:, t, :], psO[:], bob[:], op=ALU.add)
                nc.sync.dma_start(
                    ov[:, 4 * jj : 4 * jj + 4, :], osb[:, 4 * jj : 4 * jj + 4, :]
                )

            for jj in range(AC):
                for t in range(4 * jj, 4 * jj + 4):
                    psH2 = psC.tile([128, D2], f32, tag="h2", name="psH2")
                    for u in range(2):
                        nc.tensor.matmul(
                            psH2[:],
                            h1T8[:, 2 * u : 2 * u + 2, 128 * t : 128 * (t + 1)],
                            wf28[:, u, :, :],
                            start=(u == 0),
                            stop=(u == 1),
                            perf_mode=DR,
                            skip_group_check=True,
                        )
                    nc.vector.scalar_tensor_tensor(
                        xcat[:, t, :], psH2[:], 1.0 / 64.0, xcat[:, t, :],
                        op0=ALU.mult, op1=ALU.add,
                    )
                    nc.vector.bn_stats(BS3[:, t, :], xcat[:, t, :])
                    nc.vector.bn_aggr(MV3[:, t, :], BS3[:, t, :])
                sl = slice(4 * jj, 4 * jj + 4)
                newton_rsqrt(
                    RS3[:, sl], MV3[:, sl, 1], nt1[:, sl], nt2[:, sl], iters=2
                )
                if jj >= 1:
                    out_chunk(jj - 1)
            out_chunk(3)

    nc.compile()
    return nc


def _get_nc():
    if "nc" not in _CACHE:
        _CACHE["nc"] = _build_nc()
    return _CACHE["nc"]


def kernel(**inputs):
    from concourse.bass_utils import run_bass_kernel_spmd

    f = lambda k: np.asarray(inputs[k], dtype=np.float32)
    bf = lambda a: np.asarray(a, dtype=np.float32).astype(ml_dtypes.bfloat16)

    x1, x2 = f("x1"), f("x2")
    g1, b1 = f("ln1_g"), f("ln1_b")
    g2, b2 = f("ln2_g"), f("ln2_b")
    gf_, bf_ = f("lnf_g"), f("lnf_b")
    g3, b3 = f("ln3_g"), f("ln3_b")
    # fold LN gains/biases into the adjacent linear layers
    Wq = g1[:, None] * f("Wq"); bqp = b1 @ f("Wq") + f("bq")
    Wk = g2[:, None] * f("Wk"); bkp = b2 @ f("Wk") + f("bk")
    Wv1 = g1[:, None] * f("Wv1"); bv1p = b1 @ f("Wv1") + f("bv1")
    Wv2 = g2[:, None] * f("Wv2"); bv2p = b2 @ f("Wv2") + f("bv2")
    Wf1 = gf_[:, None] * f("Wf1"); bf1p = bf_ @ f("Wf1") + f("bf1")
    Wo = g3[:, None] * f("Wo"); bop = b3 @ f("Wo") + f("bo")
    Wp1, Wp2 = f("Wp1"), f("Wp2")
    bp1p = bv1p @ Wp1 + f("bp1")
    bp2p = bv2p @ Wp2 + f("bp2")

    Wf2 = f("Wf2")
    fp8 = lambda a: np.asarray(a, dtype=np.float32).astype(ml_dtypes.float8_e4m3)
    wpack = np.concatenate(
        [bf(Wq), bf(Wk), bf(Wv1), bf(Wv2), bf(Wp1), bf(Wp2),
         bf(Wo).reshape(2, 128, OUT).transpose(1, 0, 2).reshape(128, 2 * OUT)],
        axis=1,
    )
    # Wf1 [256,512] x64 -> [128, (kh, n, np)] fp8
    wf18 = fp8(
        (64.0 * Wf1).reshape(2, 128, 4, 128).transpose(1, 0, 2, 3).reshape(128, 1024)
    )
    # Wf2 [512,256] x64 -> [128, (u, s, n)] fp8
    wf28 = fp8(
        (64.0 * Wf2).reshape(2, 2, 128, D2).transpose(2, 0, 1, 3).reshape(128, 1024)
    )
    vpack = np.concatenate(
        [32.0 * bqp.reshape(1, D), 32.0 * bkp.reshape(1, D), bf1p.reshape(4, D)],
        axis=0,
    ).T.astype(np.float32)
    shared = {
        "wpack": np.ascontiguousarray(wpack),
        "wf18": np.ascontiguousarray(wf18),
        "wf28": np.ascontiguousarray(wf28),
        "vpack": np.ascontiguousarray(vpack),
        "bf2": f("bf2"), "bo": bop,
        "zpad": np.zeros((128, L), dtype=ml_dtypes.float8_e4m3),
    }

    in_maps = []
    for c in range(8):
        b, h = c // 2, c % 2
        if h == 0:
            x1c, x2c = x1[b], x2[b]
        else:
            x1c = np.concatenate([x1[b, A:], x1[b, :A]], axis=0)
            x2c = np.concatenate([x2[b, A:], x2[b, :A]], axis=0)
        tilep = lambda M, nt: np.ascontiguousarray(
            M.reshape(nt, 128, D).transpose(1, 0, 2)
        )
        m = dict(shared)
        m["x1"] = tilep(x1c, NT)
        m["x2"] = tilep(x2c, NT)
        m["res1p"] = tilep(x1c[:A] + bp1p, AT)
        m["res2p"] = tilep(x2c[:A] + bp2p, AT)
        in_maps.append(m)

    nc = _get_nc()
    res = run_bass_kernel_spmd(nc, in_maps, core_ids=list(range(8)))
    out = np.empty((B, L, OUT), np.float32)
    for c in range(8):
        b, h = c // 2, c % 2
        oc = res.results[c]["out"].transpose(1, 0, 2).reshape(A, OUT)
        out[b, h * A : (h + 1) * A, :] = oc
    return out



# revision 72
# speedup vs baseline: 1.0068x; 1.0017x over previous
"""Trainium2 Bass kernel for the dual-stream encoder block.

Sharding: 8 cores = 4 batches x 2 query-row halves (2048 rows/core).
Inputs are pre-rolled along L per core so output rows are always 0..2047;
K/V contraction uses the full 4096 rows. No cross-core communication.
"""

import sys

sys.path.insert(0, "/opt/trn_rl_repo")

import numpy as np
import ml_dtypes

B, L, D, OUT = 4, 4096, 128, 55
D2, H = 256, 512
A = 2048  # output rows per core
NT = 32  # l-tiles of 128
NG = 8  # l-groups of 4 tiles
AT = 16  # a-tiles per core
AC = 4  # a-chunks of 512
SCALE = float(1.0 / np.sqrt(np.float32(128.0)))

_CACHE = {}


def _build_nc():
    import concourse.bass as bass
    from concourse import bacc, mybir
    import concourse.tile as tile
    from concourse.masks import make_identity

    f32 = mybir.dt.float32
    bf16 = mybir.dt.bfloat16
    AF = mybir.ActivationFunctionType
    ALU = mybir.AluOpType

    fp8 = mybir.dt.float8e4

    nc = bacc.Bacc("TRN2", target_bir_lowering=False, debug=False)

    # ---- DRAM parameters -------------------------------------------------
    dx1 = nc.dram_tensor("x1", [128, NT, D], f32, kind="ExternalInput")
    dx2 = nc.dram_tensor("x2", [128, NT, D], f32, kind="ExternalInput")
    dres1 = nc.dram_tensor("res1p", [128, AT, D], f32, kind="ExternalInput")
    dres2 = nc.dram_tensor("res2p", [128, AT, D], f32, kind="ExternalInput")
    dwpack = nc.dram_tensor("wpack", [128, 878], bf16, kind="ExternalInput")
    dwf18 = nc.dram_tensor("wf18", [128, 1024], fp8, kind="ExternalInput")
    dwf28 = nc.dram_tensor("wf28", [128, 1024], fp8, kind="ExternalInput")
    dvpack = nc.dram_tensor("vpack", [128, 6], f32, kind="ExternalInput")
    dbf2 = nc.dram_tensor("bf2", [D2], f32, kind="ExternalInput")
    dbo = nc.dram_tensor("bo", [OUT], f32, kind="ExternalInput")
    dzpad = nc.dram_tensor("zpad", [128, L], fp8, kind="ExternalInput")
    dout = nc.dram_tensor("out", [128, AT, OUT], f32, kind="ExternalOutput")

    def bcast_ap(dt_handle, n):
        ap = dt_handle.ap()
        return bass.AP(tensor=ap.tensor, offset=ap.offset, ap=[[0, 128], [1, n]])

    with tile.TileContext(nc) as tc:
        import contextlib

        with contextlib.ExitStack() as ctx:
            consts = ctx.enter_context(tc.tile_pool(name="consts", bufs=1))
            big = ctx.enter_context(tc.tile_pool(name="big", bufs=1))
            stats = ctx.enter_context(tc.tile_pool(name="stats", bufs=3))
            xpool = ctx.enter_context(tc.tile_pool(name="xpool", bufs=4))
            xspool = ctx.enter_context(tc.tile_pool(name="xs", bufs=5))
            respool = ctx.enter_context(tc.tile_pool(name="res", bufs=1))
            outpool = ctx.enter_context(tc.tile_pool(name="outp", bufs=1))

            DR = mybir.MatmulPerfMode.DoubleRow

            # ---- constants ----
            ident = consts.tile([128, 128], bf16)
            make_identity(nc, ident[:])
            ones8 = consts.tile([128, 2, 1], fp8)
            nc.vector.memset(ones8[:], 1.0)

            wpk = consts.tile([128, 878], bf16)
            wf18 = consts.tile([128, 2, 4, 128], fp8)
            wf28 = consts.tile([128, 2, 2, 256], fp8)
            vpk = consts.tile([128, 6], f32)
            wq = wpk[:, 0:128]
            wk = wpk[:, 128:256]
            wv1 = wpk[:, 256:384]
            wv2 = wpk[:, 384:512]
            wp1 = wpk[:, 512:640]
            wp2 = wpk[:, 640:768]
            wov = lambda sh: wpk[:, 768 + 55 * sh : 768 + 55 * (sh + 1)]
            bq = vpk[:, 0:1]
            bk = vpk[:, 1:2]
            bf1t = vpk[:, 2:6]
            bf2b = consts.tile([128, D2], f32)
            bob = consts.tile([128, OUT], f32)

            # ---- big SBUF residents ----
            x1nT = big.tile([128, L], bf16)  # [d, l]
            x2nT = big.tile([128, L], bf16)
            q1T8 = big.tile([128, 2, L], fp8)  # [d, (real|zero), l] x32
            k2T8 = big.tile([128, 2, A], fp8)
            v1_8 = big.tile([128, NT, 128], fp8)  # [l-part, tile, d] x32
            v2_8 = big.tile([128, NT, 128], fp8)
            attF = [
                big.tile([128, NT, 512], fp8, name="attF0", tag="attF0"),
                big.tile([128, NT, 512], fp8, name="attF1", tag="attF1"),
            ]
            o1T = big.tile([128, A], bf16)  # (att @ v1)^T, unnorm, /32
            o2T = big.tile([128, A], bf16)
            invd = big.tile([128, AT], f32)
            xcat = big.tile([128, AT, D2], f32)
            xfT8 = big.tile([128, 2, A], fp8)  # lnf normalized^T x16
            x3Tl = big.tile([128, A], bf16)  # ln3 normalized^T halves
            x3Th = big.tile([128, A], bf16)
            h1T8 = big.tile([128, 4, A], fp8)

            # =========== Phase A (LN+transpose+QKV) interleaved with =====
            # =========== chunk-0 attention ===============================
            psB_cm = tc.tile_pool(name="psB", bufs=2, space="PSUM")
            psB = psB_cm.__enter__()
            psBo_cm = tc.tile_pool(name="psBo", bufs=1, space="PSUM")
            psBo = psBo_cm.__enter__()
            psA_cm = tc.tile_pool(name="psA", bufs=2, space="PSUM")
            psA = psA_cm.__enter__()

            BSs = [stats.tile([128, NT, 6], f32, tag="BSs%d" % s, name="BSs%d" % s) for s in (0, 1)]
            MVs = [stats.tile([128, NT, 2], f32, tag="MVs%d" % s, name="MVs%d" % s) for s in (0, 1)]
            RSs = [stats.tile([128, NT], f32, tag="RSs%d" % s, name="RSs%d" % s) for s in (0, 1)]
            nAs = [stats.tile([128, NT], f32, tag="nAs%d" % s, name="nAs%d" % s) for s in (0, 1)]
            nBs = [stats.tile([128, NT], f32, tag="nBs%d" % s, name="nBs%d" % s) for s in (0, 1)]
            BSf = stats.tile([128, AT, 6], f32, tag="BSf")
            MVf = stats.tile([128, AT, 2], f32, tag="MVf")
            RSf = stats.tile([128, AT], f32, tag="RSf")
            nt1 = stats.tile([128, AT], f32, tag="nt1")
            nt2 = stats.tile([128, AT], f32, tag="nt2")

            def newton_rsqrt(dst, var_ap, t1, t2, scale16=False, iters=3):
                # dst = rsqrt(var) [x16] on Pool; 1 + `iters` Newton steps,
                # var in ~[0.5, 2]; the x16 rides the last iteration free.
                nc.gpsimd.tensor_scalar(
                    dst, var_ap, -0.5, 1.5, op0=ALU.mult, op1=ALU.add
                )
                for it in range(iters):
                    last = it == iters - 1
                    c0, c1 = (-8.0, 24.0) if (last and scale16) else (-0.5, 1.5)
                    nc.gpsimd.tensor_tensor(t1, dst, dst, op=ALU.mult)
                    nc.gpsimd.tensor_tensor(t2, var_ap, t1, op=ALU.mult)
                    nc.gpsimd.tensor_scalar(
                        t1, t2, c0, c1, op0=ALU.mult, op1=ALU.add
                    )
                    nc.gpsimd.tensor_tensor(dst, dst, t1, op=ALU.mult)

            def ln_to_T(MVx, RSx, chunks, norm_eng, evac_act=False,
                        pool=None, tag="tr2"):
                # xcat rows -> x16-normalized fp8 transposed halves in xfT8
                for jj in chunks:
                    psT2 = (pool or psC).tile([128, 4, 256], bf16, tag=tag)
                    for k in range(4):
                        t = 4 * jj + k
                        xsf = xspool.tile([128, D2], bf16, tag="xsf")
                        norm_eng.tensor_scalar(
                            xsf[:],
                            xcat[:, t, :],
                            MVx[:, t, 0:1],
                            RSx[:, t : t + 1],
                            op0=ALU.subtract,
                            op1=ALU.mult,
                        )
                        nc.tensor.transpose(psT2[:, k, 0:128], xsf[:, 0:128], ident[:])
                        nc.tensor.transpose(psT2[:, k, 128:256], xsf[:, 128:256], ident[:])
                    if evac_act:
                        nc.scalar.activation(
                            xfT8[:, 0, 512 * jj : 512 * (jj + 1)],
                            psT2[:, :, 0:128], AF.Identity,
                        )
                        nc.scalar.activation(
                            xfT8[:, 1, 512 * jj : 512 * (jj + 1)],
                            psT2[:, :, 128:256], AF.Identity,
                        )
                    else:
                        nc.vector.tensor_copy(
                            xfT8[:, 0, 512 * jj : 512 * (jj + 1)], psT2[:, :, 0:128]
                        )
                        nc.vector.tensor_copy(
                            xfT8[:, 1, 512 * jj : 512 * (jj + 1)], psT2[:, :, 128:256]
                        )


            Xhold = {}
            dxs = (dx1, dx2)
            xnTs = (x1nT, x2nT)

            def s_stats(s, g):
                X = xpool.tile([128, 4, 128], f32, tag="X%d" % s)
                nc.sync.dma_start(X[:], dxs[s].ap()[:, 4 * g : 4 * g + 4, :])
                for k in range(4):
                    i = 4 * g + k
                    nc.vector.bn_stats(BSs[s][:, i, :], X[:, k, :])
                    nc.vector.bn_aggr(MVs[s][:, i, :], BSs[s][:, i, :])
                Xhold[(s, g)] = X

            def s_xform(s, g):
                X = Xhold.pop((s, g))
                for k in range(4):
                    i = 4 * g + k
                    xs = xspool.tile([128, 128], bf16, tag="xs")
                    nc.gpsimd.tensor_scalar(
                        xs[:], X[:, k, :], MVs[s][:, i, 0:1], RSs[s][:, i : i + 1],
                        op0=ALU.subtract, op1=ALU.mult,
                    )
                    if k == 0:
                        psT = psA.tile([128, 4, 128], bf16, tag="ps")
                    nc.tensor.transpose(psT[:, k, :], xs[:], ident[:])
                nc.vector.tensor_copy(
                    xnTs[s][:, 512 * g : 512 * (g + 1)], psT[:]
                )

            def qkv1(g):
                psQ = psA.tile([128, 512], f32, tag="ps")
                nc.tensor.matmul(
                    psQ[:], wq, x1nT[:, 512 * g : 512 * (g + 1)],
                    start=True, stop=True,
                )
                nc.vector.tensor_scalar(
                    q1T8[:, 0, 512 * g : 512 * (g + 1)], psQ[:], 32.0, bq,
                    op0=ALU.mult, op1=ALU.add,
                )
                psV = psA.tile([128, 512], f32, tag="ps")
                for k in range(4):
                    i = 4 * g + k
                    nc.tensor.matmul(
                        psV[:, 128 * k : 128 * (k + 1)],
                        x1nT[:, 128 * i : 128 * (i + 1)], wv1,
                        start=True, stop=True,
                    )
                nc.vector.tensor_scalar(
                    v1_8[:, 4 * g : 4 * g + 4, :], psV[:], 32.0, None, op0=ALU.mult
                )

            def qkv2(g):
                if g < 4:
                    psK = psA.tile([128, 512], f32, tag="ps")
                    nc.tensor.matmul(
                        psK[:], wk, x2nT[:, 512 * g : 512 * (g + 1)],
                        start=True, stop=True,
                    )
                    nc.vector.tensor_scalar(
                        k2T8[:, 0, 512 * g : 512 * (g + 1)], psK[:], 32.0, bk,
                        op0=ALU.mult, op1=ALU.add,
                    )
                psV = psA.tile([128, 512], f32, tag="ps")
                for k in range(4):
                    i = 4 * g + k
                    nc.tensor.matmul(
                        psV[:, 128 * k : 128 * (k + 1)],
                        x2nT[:, 128 * i : 128 * (i + 1)], wv2,
                        start=True, stop=True,
                    )
                nc.scalar.activation(
                    v2_8[:, 4 * g : 4 * g + 4, :], psV[:], AF.Identity, scale=32.0
                )

            class Chunk:
                def __init__(self, j):
                    self.j = j
                    self.psO1 = psBo.tile([128, 512], f32, tag="o1")
                    self.psO2 = psBo.tile([128, 512], f32, tag="o2")
                    self.aF = attF[j % 2]

                def att_pair(self, g, sl=slice(None)):
                    return self.aF[:, 2 * g : 2 * g + 2, sl]

                def ex_pair(self, g):
                    psE = psB.tile([128, 2, 512], f32, tag="e")
                    for k in range(2):
                        i = 2 * g + k
                        nc.tensor.matmul(
                            psE[:, k, :],
                            q1T8[:, :, 128 * i : 128 * (i + 1)],
                            k2T8[:, :, 512 * self.j : 512 * (self.j + 1)],
                            start=True, stop=True, perf_mode=DR,
                        )
                    nc.scalar.activation(
                        self.att_pair(g), psE[:], AF.Exp, scale=SCALE / 1024.0
                    )

                def av_pair(self, g):
                    ap_ = self.att_pair(g)
                    last = g == NT // 2 - 1
                    nc.tensor.matmul(
                        self.psO1[:], v1_8[:, 2 * g : 2 * g + 2, :], ap_,
                        start=(g == 0), stop=last,
                        perf_mode=DR, skip_group_check=True,
                    )
                    nc.tensor.matmul(
                        self.psO2[:], v2_8[:, 2 * g : 2 * g + 2, :], ap_,
                        start=(g == 0), stop=last,
                        perf_mode=DR, skip_group_check=True,
                    )

                def den_pair(self, g, psDen):
                    last = g == NT // 2 - 1
                    for t in range(4):
                        nc.tensor.matmul(
                            psDen[:, t : t + 1],
                            self.att_pair(g, slice(128 * t, 128 * (t + 1))),
                            ones8[:],
                            start=(g == 0), stop=last,
                            perf_mode=DR, skip_group_check=True,
                        )

            # ---- interleaved head ---------------------------------------
            # round 0 runs per-group so pair 0's chain is as short as
            # possible; later rounds batch 2 groups/stream. chunk-0 pairs
            # ride each round; chunk-1 exps lag one round.
            c0 = Chunk(0)
            c1 = Chunk(1)

            def head_group(s, g):
                s_stats(s, g)
                sl4 = slice(4 * g, 4 * g + 4)
                newton_rsqrt(
                    RSs[s][:, sl4], MVs[s][:, sl4, 1],
                    nAs[s][:, sl4], nBs[s][:, sl4], iters=2,
                )
                s_xform(s, g)
                (qkv1, qkv2)[s](g)

            s_stats(0, 0)
            # weights + zero-pad halves load right behind the first X DMA
            nc.scalar.dma_start(wpk[:], dwpack[:])
            nc.scalar.dma_start(vpk[:], dvpack[:])
            nc.sync.dma_start(q1T8[:, 1, :], dzpad[:])
            nc.sync.dma_start(k2T8[:, 1, :], dzpad.ap()[:, 0:A])
            sl4 = slice(0, 4)
            newton_rsqrt(
                RSs[0][:, sl4], MVs[0][:, sl4, 1],
                nAs[0][:, sl4], nBs[0][:, sl4], iters=2,
            )
            s_xform(0, 0)
            qkv1(0)
            head_group(1, 0)
            c0.ex_pair(0)
            c0.ex_pair(1)
            head_group(0, 1)
            head_group(1, 1)
            c0.ex_pair(2)
            c0.av_pair(0)
            c0.ex_pair(3)
            c0.av_pair(1)
            for m in range(1, 4):
                for s in (0, 1):
                    s_stats(s, 2 * m)
                    s_stats(s, 2 * m + 1)
                sl8 = slice(8 * m, 8 * m + 8)
                for s in (0, 1):
                    newton_rsqrt(
                        RSs[s][:, sl8], MVs[s][:, sl8, 1],
                        nAs[s][:, sl8], nBs[s][:, sl8], iters=2,
                    )
                for s in (0, 1):
                    s_xform(s, 2 * m)
                    s_xform(s, 2 * m + 1)
                qkv1(2 * m)
                qkv2(2 * m)
                qkv1(2 * m + 1)
                qkv2(2 * m + 1)
                for p in range(4 * m, 4 * m + 4):
                    c0.ex_pair(p)
                    c0.av_pair(p - 1)
                    c1.ex_pair(p - 4)
            c0.av_pair(NT // 2 - 1)
            for p in range(12, NT // 2):
                c1.ex_pair(p)

            psA_cm.__exit__(None, None, None)
            psD_cm = tc.tile_pool(name="psD", bufs=1, space="PSUM")
            psD = psD_cm.__enter__()
            psDen = psD.tile([128, AC, 4], f32, tag="den", name="psDen")

            r1 = respool.tile([128, AT, D], f32, tag="r1")
            r2 = respool.tile([128, AT, D], f32, tag="r2")
            r_loaded = set()

            def load_res(j):
                if j in r_loaded:
                    return
                r_loaded.add(j)
                nc.sync.dma_start(
                    r1[:, 4 * j : 4 * j + 4, :], dres1.ap()[:, 4 * j : 4 * j + 4, :]
                )
                nc.sync.dma_start(
                    r2[:, 4 * j : 4 * j + 4, :], dres2.ap()[:, 4 * j : 4 * j + 4, :]
                )

            def evac_chunk(c):
                j = c.j
                nc.vector.reciprocal(invd[:, 4 * j : 4 * j + 4], psDen[:, j, :])
                nc.vector.tensor_scalar(
                    o1T[:, 512 * j : 512 * (j + 1)], c.psO1[:], 1.0 / 32.0, None,
                    op0=ALU.mult,
                )
                nc.vector.tensor_scalar(
                    o2T[:, 512 * j : 512 * (j + 1)], c.psO2[:], 1.0 / 32.0, None,
                    op0=ALU.mult,
                )

            def proj_chunk(j):
                # output projection + residual + lnf stats for this chunk
                for k in range(4):
                    t = 4 * j + k
                    for (oT, wp, rr, off) in (
                        (o1T, wp1, r1[:, t, :], 0),
                        (o2T, wp2, r2[:, t, :], D),
                    ):
                        psP = psD.tile([128, D], f32, tag="p")
                        nc.tensor.matmul(
                            psP[:], oT[:, 128 * t : 128 * (t + 1)], wp,
                            start=True, stop=True,
                        )
                        sc = xspool.tile([128, D], f32, tag="sc")
                        nc.vector.tensor_scalar(
                            sc[:], psP[:], invd[:, t : t + 1], None, op0=ALU.mult
                        )
                        nc.gpsimd.tensor_tensor(
                            xcat[:, t, off : off + D], sc[:], rr, op=ALU.add
                        )
                    nc.vector.bn_stats(BSf[:, t, :], xcat[:, t, :])
                    nc.vector.bn_aggr(MVf[:, t, :], BSf[:, t, :])
                sl = slice(4 * j, 4 * (j + 1))
                newton_rsqrt(
                    RSf[:, sl], MVf[:, sl, 1], nt1[:, sl], nt2[:, sl], scale16=True
                )

            for g in range(NT // 2):
                c0.den_pair(g, psDen[:, 0, :])
            evac_chunk(c0)
            for g in range(NT // 2):
                c1.av_pair(g)
                c1.den_pair(g, psDen[:, 1, :])
            evac_chunk(c1)

            pend = [0, 1]
            for j in range(2, AC):
                c = Chunk(j)
                if j == 2:
                    # deferred FFN constants (transfer hides under attention)
                    nc.scalar.dma_start(wf18[:], dwf18[:])
                    nc.scalar.dma_start(wf28[:], dwf28[:])
                    nc.scalar.dma_start(bf2b[:], bcast_ap(dbf2, D2))
                    nc.scalar.dma_start(bob[:], bcast_ap(dbo, OUT))
                load_res(pend[0])
                if len(pend) > 1:
                    load_res(pend[1])
                for g in range(NT // 2):
                    c.ex_pair(g)
                    if g > 0:
                        c.av_pair(g - 1)
                        c.den_pair(g - 1, psDen[:, j, :])
                    if g in (5, 10) and pend:
                        proj_chunk(pend.pop(0))
                    hoist = {(2, 8): 0, (3, 2): 1, (3, 8): 2}.get((j, g))
                    if hoist is not None:
                        # hoist lnf transpose of a finished chunk under the
                        # exp stream (psP bank + DVE/Pool slack)
                        ln_to_T(MVf, RSf, [hoist], norm_eng=nc.vector,
                                pool=psD, tag="p")
                        for t in range(4 * hoist, 4 * hoist + 4):
                            nc.gpsimd.tensor_tensor(
                                xcat[:, t, :], xcat[:, t, :], bf2b[:],
                                op=ALU.add,
                            )
                c.av_pair(NT // 2 - 1)
                c.den_pair(NT // 2 - 1, psDen[:, j, :])
                evac_chunk(c)
                pend.append(j)
            # ---- FFN + outputs ----------------------------------------
            BS3 = stats.tile([128, AT, 6], f32, tag="BS3")
            MV3 = stats.tile([128, AT, 2], f32, tag="MV3")
            RS3 = stats.tile([128, AT], f32, tag="RS3")
            ov = dout.ap()
            osb = outpool.tile([128, AT, OUT], f32, tag="osb")

            def f1gelu(jj, pool, tag):
                for n in range(4):
                    psH = pool.tile([128, 512], f32, tag=tag, name="psH")
                    nc.tensor.matmul(
                        psH[:],
                        wf18[:, :, n, :],
                        xfT8[:, :, 512 * jj : 512 * (jj + 1)],
                        start=True,
                        stop=True,
                        perf_mode=DR,
                    )
                    nc.scalar.activation(
                        h1T8[:, n, 512 * jj : 512 * (jj + 1)],
                        psH[:],
                        AF.Gelu,
                        bias=bf1t[:, n : n + 1],
                        scale=1.0 / 1024.0,
                    )

            # gelus for hoisted chunks start right after the last exp,
            # scavenging the drained psE ring
            f1gelu(0, psB, "e")
            f1gelu(1, psB, "e")
            f1gelu(2, psB, "e")
            for j in pend:
                load_res(j)
            for j in pend:
                proj_chunk(j)
            ln_to_T(MVf, RSf, [3], norm_eng=nc.gpsimd, pool=psD, tag="p")
            for t in range(12, AT):
                nc.gpsimd.tensor_tensor(
                    xcat[:, t, :], xcat[:, t, :], bf2b[:], op=ALU.add
                )
            f1gelu(3, psB, "e")

            psD_cm.__exit__(None, None, None)
            psBo_cm.__exit__(None, None, None)
            psB_cm.__exit__(None, None, None)

            psC = ctx.enter_context(tc.tile_pool(name="psC", bufs=2, space="PSUM"))

            def out_chunk(jj):
                # ln3-normalize chunk jj (bf16) and project to the output
                psT2 = psC.tile([128, 4, 256], bf16, tag="tr2", name="psT3")
                for k in range(4):
                    t = 4 * jj + k
                    xsf = xspool.tile([128, D2], bf16, tag="xsf")
                    nc.gpsimd.tensor_scalar(
                        xsf[:], xcat[:, t, :], MV3[:, t, 0:1], RS3[:, t : t + 1],
                        op0=ALU.subtract, op1=ALU.mult,
                    )
                    nc.tensor.transpose(psT2[:, k, 0:128], xsf[:, 0:128], ident[:])
                    nc.tensor.transpose(psT2[:, k, 128:256], xsf[:, 128:256], ident[:])
                nc.scalar.activation(
                    x3Tl[:, 512 * jj : 512 * (jj + 1)], psT2[:, :, 0:128],
                    AF.Identity,
                )
                nc.vector.tensor_copy(
                    x3Th[:, 512 * jj : 512 * (jj + 1)], psT2[:, :, 128:256]
                )
                for t in range(4 * jj, 4 * jj + 4):
                    psO = psC.tile([128, OUT], f32, tag="po", name="psOu")
                    nc.tensor.matmul(
                        psO[:], x3Tl[:, 128 * t : 128 * (t + 1)], wov(0),
                        start=True, stop=False, skip_group_check=True,
                    )
                    nc.tensor.matmul(
                        psO[:], x3Th[:, 128 * t : 128 * (t + 1)], wov(1),
                        start=False, stop=True, skip_group_check=True,
                    )
                    nc.vector.tensor_tensor(osb[:, t, :], psO[:], bob[:], op=ALU.add)
                nc.sync.dma_start(
                    ov[:, 4 * jj : 4 * jj + 4, :], osb[:, 4 * jj : 4 * jj + 4, :]
                )

            for jj in range(AC):
                for t in range(4 * jj, 4 * jj + 4):
                    psH2 = psC.tile([128, D2], f32, tag="h2", name="psH2")
                    for u in range(2):
                        nc.tensor.matmul(
                            psH2[:],
                            h1T8[:, 2 * u : 2 * u + 2, 128 * t : 128 * (t + 1)],
                            wf28[:, u, :, :],
                            start=(u == 0),
                            stop=(u == 1),
                            perf_mode=DR,
                            skip_group_check=True,
                        )
                    nc.vector.scalar_tensor_tensor(
                        xcat[:, t, :], psH2[:], 1.0 / 64.0, xcat[:, t, :],
                        op0=ALU.mult, op1=ALU.add,
                    )
                    nc.vector.bn_stats(BS3[:, t, :], xcat[:, t, :])
                    nc.vector.bn_aggr(MV3[:, t, :], BS3[:, t, :])
                sl = slice(4 * jj, 4 * jj + 4)
                newton_rsqrt(
                    RS3[:, sl], MV3[:, sl, 1], nt1[:, sl], nt2[:, sl], iters=2
                )
                if jj >= 1:
                    out_chunk(jj - 1)
            out_chunk(3)

    nc.compile()
    return nc


def _get_nc():
    if "nc" not in _CACHE:
        _CACHE["nc"] = _build_nc()
    return _CACHE["nc"]


def kernel(**inputs):
    from concourse.bass_utils import run_bass_kernel_spmd

    f = lambda k: np.asarray(inputs[k], dtype=np.float32)
    bf = lambda a: np.asarray(a, dtype=np.float32).astype(ml_dtypes.bfloat16)

    x1, x2 = f("x1"), f("x2")
    g1, b1 = f("ln1_g"), f("ln1_b")
    g2, b2 = f("ln2_g"), f("ln2_b")
    gf_, bf_ = f("lnf_g"), f("lnf_b")
    g3, b3 = f("ln3_g"), f("ln3_b")
    # fold LN gains/biases into the adjacent linear layers
    Wq = g1[:, None] * f("Wq"); bqp = b1 @ f("Wq") + f("bq")
    Wk = g2[:, None] * f("Wk"); bkp = b2 @ f("Wk") + f("bk")
    Wv1 = g1[:, None] * f("Wv1"); bv1p = b1 @ f("Wv1") + f("bv1")
    Wv2 = g2[:, None] * f("Wv2"); bv2p = b2 @ f("Wv2") + f("bv2")
    Wf1 = gf_[:, None] * f("Wf1"); bf1p = bf_ @ f("Wf1") + f("bf1")
    Wo = g3[:, None] * f("Wo"); bop = b3 @ f("Wo") + f("bo")
    Wp1, Wp2 = f("Wp1"), f("Wp2")
    bp1p = bv1p @ Wp1 + f("bp1")
    bp2p = bv2p @ Wp2 + f("bp2")

    Wf2 = f("Wf2")
    fp8 = lambda a: np.asarray(a, dtype=np.float32).astype(ml_dtypes.float8_e4m3)
    wpack = np.concatenate(
        [bf(Wq), bf(Wk), bf(Wv1), bf(Wv2), bf(Wp1), bf(Wp2),
         bf(Wo).reshape(2, 128, OUT).transpose(1, 0, 2).reshape(128, 2 * OUT)],
        axis=1,
    )
    # Wf1 [256,512] x64 -> [128, (kh, n, np)] fp8
    wf18 = fp8(
        (64.0 * Wf1).reshape(2, 128, 4, 128).transpose(1, 0, 2, 3).reshape(128, 1024)
    )
    # Wf2 [512,256] x64 -> [128, (u, s, n)] fp8
    wf28 = fp8(
        (64.0 * Wf2).reshape(2, 2, 128, D2).transpose(2, 0, 1, 3).reshape(128, 1024)
    )
    vpack = np.concatenate(
        [32.0 * bqp.reshape(1, D), 32.0 * bkp.reshape(1, D), bf1p.reshape(4, D)],
        axis=0,
    ).T.astype(np.float32)
    shared = {
        "wpack": np.ascontiguousarray(wpack),
        "wf18": np.ascontiguousarray(wf18),
        "wf28": np.ascontiguousarray(wf28),
        "vpack": np.ascontiguousarray(vpack),
        "bf2": f("bf2"), "bo": bop,
        "zpad": np.zeros((128, L), dtype=ml_dtypes.float8_e4m3),
    }

    in_maps = []
    for c in range(8):
        b, h = c // 2, c % 2
        if h == 0:
            x1c, x2c = x1[b], x2[b]
        else:
            x1c = np.concatenate([x1[b, A:], x1[b, :A]], axis=0)
            x2c = np.concatenate([x2[b, A:], x2[b, :A]], axis=0)
        tilep = lambda M, nt: np.ascontiguousarray(
            M.reshape(nt, 128, D).transpose(1, 0, 2)
        )
        m = dict(shared)
        m["x1"] = tilep(x1c, NT)
        m["x2"] = tilep(x2c, NT)
        m["res1p"] = tilep(x1c[:A] + bp1p, AT)
        m["res2p"] = tilep(x2c[:A] + bp2p, AT)
        in_maps.append(m)

    nc = _get_nc()
    res = run_bass_kernel_spmd(nc, in_maps, core_ids=list(range(8)))
    out = np.empty((B, L, OUT), np.float32)
    for c in range(8):
        b, h = c // 2, c % 2
        oc = res.results[c]["out"].transpose(1, 0, 2).reshape(A, OUT)
        out[b, h * A : (h + 1) * A, :] = oc
    return out

